# revision 1
# baseline (speedup 1.0000x reference)
"""Trainium2 Bass kernel for a quantized ResNet BasicBlock (dense_cnn).

  y = relu(bn2(conv2(uq(relu(bn1(conv1(q(x), q(w1)))))), q(w2)) + x)

Strategy (8 NeuronCores, data-parallel over batch):
  - Each core processes B_LOC = B/8 images; conv weights + BN params replicated.
  - Per-tensor symmetric quantization produces small integers; they are held in
    bf16 (integers up to 256 are exact in bf16) and the 3x3 convs run on the
    tensor engine as 9 shifted matmuls (K = c_in halves of 128) accumulating in
    PSUM, so the conv arithmetic is EXACT integer arithmetic in fp32 PSUM.
  - Quantization scales then factor out of batchnorm entirely; all BN math is
    done in the integer domain with eps rescaled by 1/scale^2.
  - Global reductions (input amax, BN mean/var, post-BN max for the unsigned
    quant scale) use tiny DRAM all-reduce collectives, split per channel-group
    so the first group's collective overlaps the second group's matmuls.
  - Rounding replicates round-to-nearest-even via the +/- 1.5*2^23 trick.
  - conv1 output (Y1, integer-valued fp32) is spilled to DRAM and streamed back
    for the quantize pass; conv2 output stays in SBUF, and the epilogue for
    channel group 0 (BN2 + residual + relu + store) overlaps group 1's conv.
"""

import numpy as np
from contextlib import ExitStack

import concourse.bass as bass
import concourse.mybir as mybir
import concourse.tile as tile
import concourse.bass_isa as bass_isa
from concourse import bacc
from concourse.bass_utils import run_bass_kernel_spmd

F32 = mybir.dt.float32
BF16 = mybir.dt.bfloat16
AF = mybir.ActivationFunctionType
OP = mybir.AluOpType
AX = mybir.AxisListType

C_MAGIC = 12582912.0  # 1.5 * 2^23 : fp32 add/sub rounds to nearest-even integer
BN_EPS = 1e-5

N_CORES = 8
B = 64          # full batch
C = 256         # channels
H = W = 32
HW = H * W      # 1024
NG = 2          # channel groups of 128
NSP = 2         # spatial halves (16 rows x 32 cols = 512) per image
PHW_ = 34 * 34  # padded image size

_NC_CACHE = {}


def build_nc(b_loc=B // N_CORES, n_cores=N_CORES):
    key = (b_loc, n_cores)
    if key in _NC_CACHE:
        return _NC_CACHE[key]

    nc = bacc.Bacc("TRN2", target_bir_lowering=False, debug=False,
                   num_devices=n_cores)
    groups = [list(range(n_cores))]

    x_in = nc.dram_tensor("x", [b_loc, C, H, W], F32, kind="ExternalInput").ap()
    w1t = nc.dram_tensor("w1t", [9, C, C], F32, kind="ExternalInput").ap()
    w2t = nc.dram_tensor("w2t", [9, C, C], F32, kind="ExternalInput").ap()
    gamma1 = nc.dram_tensor("gamma1", [C], F32, kind="ExternalInput").ap()
    beta1 = nc.dram_tensor("beta1", [C], F32, kind="ExternalInput").ap()
    gamma2 = nc.dram_tensor("gamma2", [C], F32, kind="ExternalInput").ap()
    beta2 = nc.dram_tensor("beta2", [C], F32, kind="ExternalInput").ap()
    out = nc.dram_tensor("out", [b_loc, C, H, W], F32, kind="ExternalOutput").ap()

    wts = [w1t, w2t]
    NT = b_loc * NSP          # psum tiles per c_out group per conv

    with tile.TileContext(nc) as tc, ExitStack() as ctx:
        per = ctx.enter_context(tc.tile_pool(name="persist", bufs=1))
        wrot = ctx.enter_context(tc.tile_pool(name="wrot", bufs=4))
        wzrot = ctx.enter_context(tc.tile_pool(name="wzrot", bufs=2))
        bigin = ctx.enter_context(tc.tile_pool(name="bigin", bufs=3))
        zbig = ctx.enter_context(tc.tile_pool(name="zbig", bufs=3))
        y1rot = ctx.enter_context(tc.tile_pool(name="y1rot", bufs=3))
        xrrot = ctx.enter_context(tc.tile_pool(name="xrrot", bufs=2))
        orot = ctx.enter_context(tc.tile_pool(name="orot", bufs=2))
        trot = ctx.enter_context(tc.tile_pool(name="trot", bufs=2))
        psum = ctx.enter_context(tc.tile_pool(name="psum", bufs=8, space="PSUM"))
        dram = ctx.enter_context(tc.tile_pool(name="dram", bufs=1, space="DRAM"))

        def pt(shape, dtype, name):
            return per.tile(shape, dtype, tag=name, name=name)

        def vts(outap, inap, s1, s2=None, op0=OP.mult, op1=None):
            if op1 is None:
                nc.vector.tensor_scalar(outap, inap, s1, None, op0=op0)
            else:
                nc.vector.tensor_scalar(outap, inap, s1, s2, op0=op0, op1=op1)

        # padded quantized input tiles (memset FIRST so the gpsimd queue is
        # not blocked behind collective triggers; reused later as conv2 input)
        xpad = [[None] * b_loc for _ in range(NG)]
        xp3 = [[None] * b_loc for _ in range(NG)]
        for g in range(NG):
            for i in range(b_loc):
                t = pt([128, PHW_], BF16, f"xpad{g}_{i}")
                nc.gpsimd.memset(t[:], 0.0)
                xpad[g][i] = t
                xp3[g][i] = t.rearrange("p (h w) -> p h w", w=34)

        # ---------- warmup collective: absorb comm-channel init early ------
        wu = pt([128, 1], F32, "wu")
        nc.vector.memset(wu[:], 0.0)
        wu_in = dram.tile([128], F32, tag="wu_in", name="wu_in")
        wu_out = dram.tile([128], F32, tag="wu_out", name="wu_out")
        nc.sync.dma_start(wu_in[:].rearrange("(c u) -> c u", u=1), wu[:])
        nc.gpsimd.collective_compute("AllReduce", OP.max, replica_groups=groups,
                                     ins=[wu_in.opt()], outs=[wu_out.opt()])

        # ---------- constants ----------
        cmag = pt([128, 1], F32, "cmag")
        nc.vector.memset(cmag[:], C_MAGIC)

        # gamma/beta as [128, o] vectors
        gb = {}
        for name, t in (("g1", gamma1), ("b1", beta1), ("g2", gamma2), ("b2", beta2)):
            v = pt([128, NG], F32, f"gb_{name}")
            for o in range(NG):
                nc.sync.dma_start(
                    v[:, o:o + 1],
                    t[o * 128:(o + 1) * 128].rearrange("(c u) -> c u", u=1))
            gb[name] = v

        # ---------- phase A: local amax of x -> C0 collective ----------
        # one [128, 2*HW] DMA + one XY-reduce per image (fewer DMA<->DVE
        # handshakes -> C0 fires earlier on every core)
        xamax = pt([128, b_loc], F32, "xamax")
        for i in range(b_loc):
            xin = bigin.tile([128, NG * HW], F32, tag="bigin", name="bigin")
            nc.sync.dma_start(
                xin[:].rearrange("c (g hw) -> c g hw", g=NG),
                x_in[i].rearrange("(g c) h w -> c g (h w)", c=128))
            nc.vector.tensor_reduce(
                xamax[:, i:i + 1], xin[:].rearrange("c (g hw) -> c g hw", g=NG),
                axis=AX.XY, op=OP.max, apply_absolute_value=True)
        lamax = pt([128, 1], F32, "lamax")
        nc.vector.tensor_reduce(lamax[:], xamax[:], axis=AX.X, op=OP.max)
        cc0_in = dram.tile([128], F32, tag="cc0_in", name="cc0_in")
        cc0_out = dram.tile([128], F32, tag="cc0_out", name="cc0_out")
        nc.sync.dma_start(cc0_in[:].rearrange("(c u) -> c u", u=1), lamax[:])
        nc.gpsimd.collective_compute("AllReduce", OP.max, replica_groups=groups,
                                     ins=[cc0_in.opt()], outs=[cc0_out.opt()])

        # weights: amax pass (overlaps the collective)
        rw = []
        for ci_, wt in enumerate(wts):
            wamax = pt([128, NG * 9], F32, f"wamax{ci_}")
            for g in range(NG):
                for k in range(9):
                    wr = wrot.tile([128, C], F32, tag="wrot", name="wrot")
                    nc.sync.dma_start(wr[:], wt[k, g * 128:(g + 1) * 128, :])
                    nc.vector.tensor_reduce(
                        wamax[:, g * 9 + k: g * 9 + k + 1], wr[:],
                        axis=AX.X, op=OP.max, apply_absolute_value=True)
            wl = pt([128, 1], F32, f"wlmax{ci_}")
            nc.vector.tensor_reduce(wl[:], wamax[:], axis=AX.X, op=OP.max)
            # cross-partition max + broadcast via DMA gather/scatter + DVE
            # (avoids gpsimd, whose queue is blocked behind collective triggers)
            wlt = pt([1, 128], F32, f"wlt{ci_}")
            nc.sync.dma_start(wlt[:], wl[:])
            wred = pt([1, 1], F32, f"wred{ci_}")
            nc.vector.tensor_reduce(wred[:], wlt[:], axis=AX.X, op=OP.max)
            wrep = pt([1, 128], F32, f"wrep{ci_}")
            nc.vector.tensor_scalar(wrep[:], wlt[:], wred[:, 0:1], None,
                                    op0=OP.max)
            gw = pt([128, 1], F32, f"gwmax{ci_}")
            nc.sync.dma_start(gw[:], wrep[:])
            sw = pt([128, 1], F32, f"sw{ci_}")
            vts(sw[:], gw[:], 1.0 / 127.0, 1e-12, op0=OP.mult, op1=OP.add)
            rwv = pt([128, 1], F32, f"rw{ci_}")
            nc.vector.reciprocal(rwv[:], sw[:])
            rw.append((sw, rwv))

        # ---------- phase B: quantize weights + x ----------
        wq = []  # wq[conv][g] : [128, 9*256] bf16, block k at k*256
        for ci_, wt in enumerate(wts):
            wqc = []
            for g in range(NG):
                wqg = pt([128, 9 * C], BF16, f"wq{ci_}_{g}")
                for k in range(9):
                    wr = wrot.tile([128, C], F32, tag="wrot", name="wrot")
                    nc.sync.dma_start(wr[:], wt[k, g * 128:(g + 1) * 128, :])
                    wz = wzrot.tile([128, C], F32, tag="wzrot", name="wzrot")
                    nc.vector.tensor_scalar(wz[:], wr[:], rw[ci_][1][:, 0:1],
                                            C_MAGIC, op0=OP.mult, op1=OP.add)
                    vts(wqg[:, k * C:(k + 1) * C], wz[:], -C_MAGIC, op0=OP.add)
                wqc.append(wqg)
            wq.append(wqc)

        # C0 result -> global x scale (DMA gather + DVE + DMA scatter; no
        # gpsimd: PartitionAllReduce forces a ~8us ucode LIBRARY_RELOAD)
        gx1 = pt([1, 128], F32, "gx1")
        nc.sync.dma_start(gx1[:], cc0_out[:].rearrange("(u c) -> u c", u=1))
        gxr = pt([1, 1], F32, "gxr")
        nc.vector.tensor_reduce(gxr[:], gx1[:], axis=AX.X, op=OP.max)
        gxp = pt([1, 128], F32, "gxp")
        nc.vector.tensor_scalar(gxp[:], gx1[:], gxr[:, 0:1], None, op0=OP.max)
        gxamax = pt([128, 1], F32, "gxamax")
        nc.sync.dma_start(gxamax[:], gxp[:])
        sx = pt([128, 1], F32, "sx")
        vts(sx[:], gxamax[:], 1.0 / 127.0, 1e-12, op0=OP.mult, op1=OP.add)
        rx = pt([128, 1], F32, "rx")
        nc.vector.reciprocal(rx[:], sx[:])

        for i in range(b_loc):
            for g in range(NG):
                xin = bigin.tile([128, HW], F32, tag="bigin", name="bigin")
                nc.sync.dma_start(xin[:], x_in[i, g * 128:(g + 1) * 128, :, :])
                zx = zbig.tile([128, HW], F32, tag="zbig", name="zbig")
                nc.scalar.activation(zx[:], xin[:], AF.Identity,
                                     bias=cmag[:, 0:1], scale=rx[:, 0:1])
                vts(xp3[g][i][:, 1:33, 1:33],
                    zx[:].rearrange("p (h w) -> p h w", w=32), -C_MAGIC, op0=OP.add)

        # ---------- conv helper: one c_out group ----------
        # weight-outer over groups of GT psum tiles: each stationary weight is
        # loaded once per GT matmuls, amortizing LDWEIGHTS.
        GT = 4
        def conv_group(o, wqc, post_tile):
            pairs = [(i, s) for i in range(b_loc) for s in range(NSP)]
            for g0 in range(0, len(pairs), GT):
                grp = pairs[g0:g0 + GT]
                pss = [psum.tile([128, 512], F32, tag="ps", name="ps")
                       for _ in grp]
                for g in range(NG):
                    for k in range(9):
                        ky, kx = divmod(k, 3)
                        first = (g == 0) and (k == 0)
                        last = (g == NG - 1) and (k == 8)
                        wslice = wqc[g][:, k * C + o * 128: k * C + o * 128 + 128]
                        for t, (i, s) in enumerate(grp):
                            nc.tensor.matmul(
                                pss[t][:], wslice,
                                xp3[g][i][:, s * 16 + ky: s * 16 + ky + 16,
                                          kx: kx + 32],
                                start=first, stop=last)
                for t, (i, s) in enumerate(grp):
                    post_tile(i, s, i * NSP + s, pss[t])

        def mk_stat_payload(bnb, tagp):
            """bn_aggr over all tiles -> [mean, var+mean^2] payload [128,2]"""
            a = pt([128, 2], F32, f"agg{tagp}")
            nc.vector.bn_aggr(a[:], bnb[:])
            pay = pt([128, 2], F32, f"pays{tagp}")
            nc.vector.tensor_copy(pay[:, 0:1], a[:, 0:1])
            m2 = pt([128, 1], F32, f"m2{tagp}")
            vts(m2[:], a[:, 0:1], a[:, 0:1], op0=OP.mult)
            nc.vector.tensor_add(pay[:, 1:2], m2[:], a[:, 1:2])
            return pay

        def all_gather(pay, tagp):
            """AllGather [128,S] -> SBUF view [128, S, n_cores] (strided)"""
            S = pay.shape[1]
            cin = dram.tile([128, S], F32, tag=f"cg{tagp}_in", name=f"cg{tagp}_in")
            cout = dram.tile([n_cores, 128, S], F32, tag=f"cg{tagp}_out",
                             name=f"cg{tagp}_out")
            nc.sync.dma_start(cin[:], pay[:])
            nc.gpsimd.collective_compute("AllGather", OP.bypass,
                                         replica_groups=groups,
                                         ins=[cin.opt()], outs=[cout.opt()])
            res = pt([128, n_cores * S], F32, f"cg{tagp}_res")
            nc.gpsimd.dma_start(
                res[:].rearrange("c (r s) -> c r s", s=S),
                cout[:].rearrange("r c s -> c r s"))
            return res.rearrange("c (r s) -> c s r", s=S)

        def all_reduce(pay, op, tagp):
            cin = dram.tile([128, pay.shape[1]], F32, tag=f"cc{tagp}_in",
                            name=f"cc{tagp}_in")
            cout = dram.tile([128, pay.shape[1]], F32, tag=f"cc{tagp}_out",
                             name=f"cc{tagp}_out")
            nc.sync.dma_start(cin[:], pay[:])
            nc.gpsimd.collective_compute("AllReduce", op, replica_groups=groups,
                                         ins=[cin.opt()], outs=[cout.opt()])
            res = pt([128, pay.shape[1]], F32, f"cc{tagp}_res")
            nc.gpsimd.dma_start(res[:], cout[:])
            return res

        def bn_coeffs(gsum, s_parts, gam, bet, tag):
            """global [mean, E[x^2]] sums over cores -> A, B  (t = A*Y + B)"""
            mean = pt([128, 1], F32, f"mean{tag}")
            vts(mean[:], gsum[:, 0:1], 1.0 / n_cores, op0=OP.mult)
            e2 = pt([128, 1], F32, f"e2{tag}")
            vts(e2[:], gsum[:, 1:2], 1.0 / n_cores, op0=OP.mult)
            m2g = pt([128, 1], F32, f"m2g{tag}")
            vts(m2g[:], mean[:], mean[:, 0:1], op0=OP.mult)
            var = pt([128, 1], F32, f"var{tag}")
            nc.vector.tensor_sub(var[:], e2[:], m2g[:])
            se = pt([128, 1], F32, f"se{tag}")
            vts(se[:], s_parts[0][:], s_parts[1][:, 0:1], op0=OP.mult)
            se2 = pt([128, 1], F32, f"se2{tag}")
            vts(se2[:], se[:], se[:, 0:1], op0=OP.mult)
            se2r = pt([128, 1], F32, f"se2r{tag}")
            nc.vector.reciprocal(se2r[:], se2[:])
            epse = pt([128, 1], F32, f"epse{tag}")
            vts(epse[:], se2r[:], float(BN_EPS), op0=OP.mult)
            std = pt([128, 1], F32, f"std{tag}")
            nc.scalar.activation(std[:], var[:], AF.Sqrt, bias=epse[:, 0:1], scale=1.0)
            stdr = pt([128, 1], F32, f"stdr{tag}")
            nc.vector.reciprocal(stdr[:], std[:])
            A = pt([128, 1], F32, f"A{tag}")
            vts(A[:], gam[:], stdr[:, 0:1], op0=OP.mult)
            negmA = pt([128, 1], F32, f"negmA{tag}")
            vts(negmA[:], mean[:], A[:, 0:1], -1.0, op0=OP.mult, op1=OP.mult)
            Bv = pt([128, 1], F32, f"B{tag}")
            nc.vector.tensor_add(Bv[:], negmA[:], bet[:])
            return A, Bv

        # ---------- phase C: conv1 (per group: matmuls then stats collective) --
        y1d = dram.tile([NG, b_loc, 128, HW], F32, tag="y1d", name="y1d")
        gs1, gm1 = [], []
        A1, B1, tmx = [], [], []
        for o in range(NG):
            bnb = pt([128, 6 * NT], F32, f"bnb1_{o}")
            chmx = pt([128, NT], F32, f"chmx1_{o}")
            chmn = pt([128, NT], F32, f"chmn1_{o}")

            def post1(i, s, t, ps, bnb=bnb, chmx=chmx, chmn=chmn, o=o):
                y1sb = y1rot.tile([128, 512], F32, tag="y1rot", name="y1rot")
                nc.scalar.copy(y1sb[:], ps[:])
                nc.sync.dma_start(y1d[o, i, :, s * 512:(s + 1) * 512], y1sb[:])
                nc.vector.bn_stats(bnb[:, 6 * t: 6 * t + 6], ps[:])
                nc.vector.tensor_reduce(chmx[:, t:t + 1], ps[:], axis=AX.X, op=OP.max)
                nc.vector.tensor_reduce(chmn[:, t:t + 1], ps[:], axis=AX.X, op=OP.min)

            conv_group(o, wq[0], post1)
            # AllGather carries [mean, var+mean^2, chmax, -chmin]; for the
            # last group, split into two partial rounds over tile halves so
            # the first round re-syncs cores mid-phase and only a drift-free
            # short round stays exposed at the end.
            halves = ([(0, NT)] if o == 0 else
                      [(0, NT // 2), (NT // 2, NT)])
            gss, gms = [], []
            for hh, (ta, tb) in enumerate(halves):
                a = pt([128, 2], F32, f"agg1_{o}_{hh}")
                nc.vector.bn_aggr(a[:], bnb[:, 6 * ta: 6 * tb])
                pay = pt([128, 4], F32, f"pay1_{o}_{hh}")
                nc.vector.tensor_copy(pay[:, 0:1], a[:, 0:1])
                m2 = pt([128, 1], F32, f"m2_1_{o}_{hh}")
                vts(m2[:], a[:, 0:1], a[:, 0:1], op0=OP.mult)
                nc.vector.tensor_add(pay[:, 1:2], m2[:], a[:, 1:2])
                nc.vector.tensor_reduce(pay[:, 2:3], chmx[:, ta:tb],
                                        axis=AX.X, op=OP.max)
                mn = pt([128, 1], F32, f"mn1_{o}_{hh}")
                nc.vector.tensor_reduce(mn[:], chmn[:, ta:tb],
                                        axis=AX.X, op=OP.min)
                vts(pay[:, 3:4], mn[:], -1.0, op0=OP.mult)
                gv = all_gather(pay, f"1_{o}_{hh}")  # [128, 4, n_cores]
                gsh = pt([128, 2], F32, f"gs1_{o}_{hh}")
                nc.vector.tensor_reduce(gsh[:], gv[:, 0:2, :], axis=AX.X, op=OP.add)
                gmh = pt([128, 2], F32, f"gm1_{o}_{hh}")
                nc.vector.tensor_reduce(gmh[:], gv[:, 2:4, :], axis=AX.X, op=OP.max)
                gss.append(gsh)
                gms.append(gmh)
            if len(gss) == 1:
                gs, gm = gss[0], gms[0]
            else:
                gs = pt([128, 2], F32, f"gs1_{o}")
                nc.vector.tensor_add(gs[:], gss[0][:], gss[1][:])
                vts(gs[:], gs[:], 0.5, op0=OP.mult)
                gm = pt([128, 2], F32, f"gm1_{o}")
                nc.vector.tensor_max(gm[:], gms[0][:], gms[1][:])
            gs1.append(gs)
            gm1.append(gm)
            # BN1 coefficients for this group (overlaps the other group's conv)
            a_, b_ = bn_coeffs(gs, (sx, rw[0][0]), gb["g1"][:, o:o + 1],
                               gb["b1"][:, o:o + 1], f"1_{o}")
            A1.append(a_)
            B1.append(b_)
            c1 = pt([128, 1], F32, f"c1_{o}")
            vts(c1[:], gm[:, 0:1], a_[:, 0:1], b_[:, 0:1], op0=OP.mult, op1=OP.add)
            mnv = pt([128, 1], F32, f"mnv_{o}")
            vts(mnv[:], gm[:, 1:2], -1.0, op0=OP.mult)
            c2 = pt([128, 1], F32, f"c2_{o}")
            vts(c2[:], mnv[:], a_[:, 0:1], b_[:, 0:1], op0=OP.mult, op1=OP.add)
            tm = pt([128, 1], F32, f"tmx_{o}")
            nc.vector.tensor_max(tm[:], c1[:], c2[:])
            tmx.append(tm)

        # ---------- phase D: unsigned quant scale ----------
        tmall = pt([128, 1], F32, "tmall")
        nc.vector.tensor_max(tmall[:], tmx[0][:], tmx[1][:])
        vts(tmall[:], tmall[:], 0.0, op0=OP.max)
        tgt = pt([1, 128], F32, "tgt")
        nc.sync.dma_start(tgt[:], tmall[:])
        tgr = pt([1, 1], F32, "tgr")
        nc.vector.tensor_reduce(tgr[:], tgt[:], axis=AX.X, op=OP.max)
        tgp = pt([1, 128], F32, "tgp")
        nc.vector.tensor_scalar(tgp[:], tgt[:], tgr[:, 0:1], None, op0=OP.max)
        tg = pt([128, 1], F32, "tg")
        nc.sync.dma_start(tg[:], tgp[:])
        s2q = pt([128, 1], F32, "s2q")
        vts(s2q[:], tg[:], 1.0 / 255.0, 1e-12, op0=OP.mult, op1=OP.add)
        r2q = pt([128, 1], F32, "r2q")
        nc.vector.reciprocal(r2q[:], s2q[:])
        A1p, B1p = [], []
        for o in range(NG):
            ap_ = pt([128, 1], F32, f"A1p_{o}")
            vts(ap_[:], A1[o][:], r2q[:, 0:1], op0=OP.mult)
            bp_ = pt([128, 1], F32, f"B1p_{o}")
            vts(bp_[:], B1[o][:], r2q[:, 0:1], op0=OP.mult)
            A1p.append(ap_)
            B1p.append(bp_)

        # ---------- phase E: quantize Y1 -> q (into xpad buffers) ----------
        # q = relu(round(A1p*Y + B1p)); round via ACT +C then DVE -C with relu
        for i in range(b_loc):
            for g in range(NG):
                y1in = bigin.tile([128, HW], F32, tag="bigin", name="bigin")
                nc.sync.dma_start(y1in[:], y1d[g, i, :, :])
                z1 = zbig.tile([128, HW], F32, tag="zbig", name="zbig")
                nc.scalar.activation(z1[:], y1in[:], AF.Identity,
                                     bias=B1p[g][:, 0:1], scale=A1p[g][:, 0:1])
                z2 = zbig.tile([128, HW], F32, tag="zbig", name="zbig")
                nc.scalar.activation(z2[:], z1[:], AF.Identity,
                                     bias=cmag[:, 0:1], scale=1.0)
                nc.vector.tensor_scalar(
                    xp3[g][i][:, 1:33, 1:33],
                    z2[:].rearrange("p (h w) -> p h w", w=32),
                    -C_MAGIC, 0.0, op0=OP.add, op1=OP.max)

        # ---------- phase F/G/H: conv2 per group + BN2 + final epilogue ------
        for o in range(NG):
            y2 = pt([128, NT * 512], F32, f"y2_{o}")
            bnb = pt([128, 6 * NT], F32, f"bnb2_{o}")

            def post2(i, s, t, ps, y2=y2, bnb=bnb):
                nc.scalar.copy(y2[:, t * 512:(t + 1) * 512], ps[:])
                nc.vector.bn_stats(bnb[:, 6 * t: 6 * t + 6], ps[:])

            conv_group(o, wq[1], post2)
            if o == 0:
                pays = mk_stat_payload(bnb, f"2_{o}")
                gs2 = all_reduce(pays, OP.add, f"2a{o}")
            else:
                pa = mk_stat_payload(bnb[:, 0:6 * (NT // 2)], f"2_{o}_a")
                pb = mk_stat_payload(bnb[:, 6 * (NT // 2):], f"2_{o}_b")
                ga = all_reduce(pa, OP.add, f"2a{o}a")
                gb_ = all_reduce(pb, OP.add, f"2a{o}b")
                gs2 = pt([128, 2], F32, f"gs2_{o}")
                nc.vector.tensor_add(gs2[:], ga[:], gb_[:])
                vts(gs2[:], gs2[:], 0.5, op0=OP.mult)
            A2, B2 = bn_coeffs(gs2, (s2q, rw[1][0]), gb["g2"][:, o:o + 1],
                               gb["b2"][:, o:o + 1], f"2_{o}")
            # final: relu(A2*Y2 + B2 + x), one [128,1024] tile per image
            for i in range(b_loc):
                xres = xrrot.tile([128, HW], F32, tag="xrrot", name="xrrot")
                nc.sync.dma_start(xres[:], x_in[i, o * 128:(o + 1) * 128, :, :])
                tt = trot.tile([128, HW], F32, tag="trot", name="trot")
                nc.vector.scalar_tensor_tensor(
                    tt[:], y2[:, i * HW:(i + 1) * HW], A2[:, 0:1],
                    xres[:], op0=OP.mult, op1=OP.add)
                osb = orot.tile([128, HW], F32, tag="orot", name="orot")
                nc.scalar.activation(osb[:], tt[:], AF.Relu,
                                     bias=B2[:, 0:1], scale=1.0)
                nc.sync.dma_start(out[i, o * 128:(o + 1) * 128, :, :], osb[:])

    nc.compile()
    _NC_CACHE[key] = nc
    return nc


def _prep_host(x, w1, w2, gamma1, beta1, gamma2, beta2, n_cores):
    w1t = np.ascontiguousarray(
        np.transpose(np.asarray(w1, np.float32), (2, 3, 1, 0)).reshape(9, C, C))
    w2t = np.ascontiguousarray(
        np.transpose(np.asarray(w2, np.float32), (2, 3, 1, 0)).reshape(9, C, C))
    x = np.ascontiguousarray(np.asarray(x, np.float32))
    b_loc = x.shape[0] // n_cores
    in_maps = []
    for c in range(n_cores):
        in_maps.append({
            "x": x[c * b_loc:(c + 1) * b_loc],
            "w1t": w1t, "w2t": w2t,
            "gamma1": np.asarray(gamma1, np.float32),
            "beta1": np.asarray(beta1, np.float32),
            "gamma2": np.asarray(gamma2, np.float32),
            "beta2": np.asarray(beta2, np.float32),
        })
    return in_maps, b_loc


def kernel(x, w1, gamma1, beta1, w2, gamma2, beta2, _trace=False):
    in_maps, b_loc = _prep_host(x, w1, w2, gamma1, beta1, gamma2, beta2, N_CORES)
    nc = build_nc(b_loc, N_CORES)
    res = run_bass_kernel_spmd(nc, in_maps, list(range(N_CORES)), trace=_trace)
    out = np.concatenate(
        [np.asarray(res.results[c]["out"]).reshape(b_loc, C, H, W)
         for c in range(N_CORES)], axis=0)
    if _trace:
        kernel._last_results = res
    return out



# revision 4
# speedup vs baseline: 1.0679x; 1.0679x over previous
"""Trainium2 Bass kernel for a quantized ResNet BasicBlock (dense_cnn).

  y = relu(bn2(conv2(uq(relu(bn1(conv1(q(x), q(w1)))))), q(w2)) + x)

Strategy (8 NeuronCores, data-parallel over batch):
  - Each core processes B_LOC = B/8 images; conv weights + BN params replicated.
  - Per-tensor symmetric quantization produces small integers held in bf16
    (ints <= 256 exact); 3x3 convs run as 9 shifted matmuls per c_in group
    accumulating in fp32 PSUM => exact integer arithmetic.
  - Quant scales factor out of batchnorm; all BN math in the integer domain.
  - v2 layout: x is loaded ONCE into SBUF (XY tiles) and never re-read for the
    quantize pass; conv1 output Y1 lives in the same SBUF tiles (x is dead
    after quantize), conv2 output Y2 again reuses them.  The residual is
    prefetched into SBUF during the convs.  No DRAM spill/reload of Y1.
  - Collectives: warmup AllReduce triggered at t~0 (absorbs comm-channel
    init), AllGather everywhere else (shorter mesh than AllReduce); per-group
    stats split [0:12]/[12:16] with conv tail groups [2,1,1] so only a 4-tile
    payload is exposed after the last matmul; collective payload DMAs ride
    the gpsimd queue (never stuck behind bulk traffic on the sync queue).
  - Rounding replicates round-to-nearest-even via the +/- 1.5*2^23 trick.
"""

import numpy as np
from contextlib import ExitStack

import concourse.bass as bass
import concourse.mybir as mybir
import concourse.tile as tile
import concourse.bass_isa as bass_isa
from concourse import bacc
from concourse.bass_utils import run_bass_kernel_spmd

F32 = mybir.dt.float32
BF16 = mybir.dt.bfloat16
AF = mybir.ActivationFunctionType
OP = mybir.AluOpType
AX = mybir.AxisListType

C_MAGIC = 12582912.0  # 1.5 * 2^23 : fp32 add/sub rounds to nearest-even integer
BN_EPS = 1e-5

N_CORES = 8
B = 64          # full batch
C = 256         # channels
H = W = 32
HW = H * W      # 1024
NG = 2          # channel groups of 128
NSP = 2         # spatial halves (16 rows x 32 cols = 512) per image
PHW_ = 34 * 34  # padded image size

_NC_CACHE = {}


def build_nc(b_loc=B // N_CORES, n_cores=N_CORES):
    key = (b_loc, n_cores)
    if key in _NC_CACHE:
        return _NC_CACHE[key]

    nc = bacc.Bacc("TRN2", target_bir_lowering=False, debug=False,
                   num_devices=n_cores)
    groups = [list(range(n_cores))]

    x_in = nc.dram_tensor("x", [b_loc, C, H, W], F32, kind="ExternalInput").ap()
    w1t = nc.dram_tensor("w1t", [9, C, C], F32, kind="ExternalInput").ap()
    w2t = nc.dram_tensor("w2t", [9, C, C], F32, kind="ExternalInput").ap()
    gamma1 = nc.dram_tensor("gamma1", [C], F32, kind="ExternalInput").ap()
    beta1 = nc.dram_tensor("beta1", [C], F32, kind="ExternalInput").ap()
    gamma2 = nc.dram_tensor("gamma2", [C], F32, kind="ExternalInput").ap()
    beta2 = nc.dram_tensor("beta2", [C], F32, kind="ExternalInput").ap()
    out = nc.dram_tensor("out", [b_loc, C, H, W], F32, kind="ExternalOutput").ap()

    wts = [w1t, w2t]
    NT = b_loc * NSP          # psum tiles per c_out group per conv (16)

    with tile.TileContext(nc) as tc, ExitStack() as ctx:
        per = ctx.enter_context(tc.tile_pool(name="persist", bufs=1))
        wf32 = ctx.enter_context(tc.tile_pool(name="wf32", bufs=2))
        zrot = ctx.enter_context(tc.tile_pool(name="zrot", bufs=2))
        orot = ctx.enter_context(tc.tile_pool(name="orot", bufs=2))
        trot = ctx.enter_context(tc.tile_pool(name="trot", bufs=2))
        psum = ctx.enter_context(tc.tile_pool(name="psum", bufs=8, space="PSUM"))
        dram = ctx.enter_context(tc.tile_pool(name="dram", bufs=1, space="DRAM"))

        def pt(shape, dtype, name):
            return per.tile(shape, dtype, tag=name, name=name)

        def vts(outap, inap, s1, s2=None, op0=OP.mult, op1=None):
            if op1 is None:
                nc.vector.tensor_scalar(outap, inap, s1, None, op0=op0)
            else:
                nc.vector.tensor_scalar(outap, inap, s1, s2, op0=op0, op1=op1)

        # ---------- warmup collective: first thing on the gpsimd queue ------
        wu = pt([128, 1], F32, "wu")
        nc.vector.memset(wu[:], 0.0)
        wu_in = dram.tile([128], F32, tag="wu_in", name="wu_in")
        wu_out = dram.tile([128], F32, tag="wu_out", name="wu_out")
        nc.gpsimd.dma_start(wu_in[:].rearrange("(c u) -> c u", u=1), wu[:])
        nc.gpsimd.collective_compute("AllReduce", OP.max, replica_groups=groups,
                                     ins=[wu_in.opt()], outs=[wu_out.opt()])

        # padded quantized input tiles; memset on DVE (gpsimd queue must stay
        # free so the warmup collective triggers at t~0)
        xpad = [[None] * b_loc for _ in range(NG)]
        xp3 = [[None] * b_loc for _ in range(NG)]
        for g in range(NG):
            for i in range(b_loc):
                t = pt([128, PHW_], BF16, f"xpad{g}_{i}")
                nc.vector.memset(t[:], 0.0)
                xpad[g][i] = t
                xp3[g][i] = t.rearrange("p (h w) -> p h w", w=34)

        # ---------- constants ----------
        cmag = pt([128, 1], F32, "cmag")
        nc.vector.memset(cmag[:], C_MAGIC)

        # ---------- phase A: x -> SBUF (kept!), local amax -> C0 AllGather --
        XY = [pt([128, (b_loc // 2) * 2048], F32, f"XY{h}") for h in range(2)]

        def xcols(i):      # x image i lives in XY[i//4] cols (i%4)*2048
            return XY[i // (b_loc // 2)], (i % (b_loc // 2)) * 2048

        def ycols(o, i, s):  # conv output (o,i,s) -> XY[o] cols i*1024+s*512
            return XY[o], i * 1024 + s * 512

        xamax = pt([128, b_loc], F32, "xamax")
        for i in range(b_loc):
            xt, c0 = xcols(i)
            nc.sync.dma_start(
                xt[:, c0:c0 + 2048].rearrange("c (g hw) -> c g hw", g=NG),
                x_in[i].rearrange("(g c) h w -> c g (h w)", c=128))
            nc.vector.tensor_reduce(
                xamax[:, i:i + 1],
                xt[:, c0:c0 + 2048].rearrange("c (g hw) -> c g hw", g=NG),
                axis=AX.XY, op=OP.max, apply_absolute_value=True)
        lamax = pt([128, 1], F32, "lamax")
        nc.vector.tensor_reduce(lamax[:], xamax[:], axis=AX.X, op=OP.max)
        # cross-partition max BEFORE the collective (in slack time): the
        # C0 readback then needs no transpose hop on the critical path.
        lat = pt([1, 128], F32, "lat")
        nc.sync.dma_start(lat[:], lamax[:])
        lar = pt([1, 1], F32, "lar")
        nc.vector.tensor_reduce(lar[:], lat[:], axis=AX.X, op=OP.max)
        lap = pt([1, 128], F32, "lap")
        nc.vector.tensor_scalar(lap[:], lat[:], lar[:, 0:1], None, op0=OP.max)
        gxl = pt([128, 1], F32, "gxl")
        nc.sync.dma_start(gxl[:], lap[:])
        cc0_in = dram.tile([128], F32, tag="cc0_in", name="cc0_in")
        cc0_out = dram.tile([n_cores, 128], F32, tag="cc0_out", name="cc0_out")
        nc.gpsimd.dma_start(cc0_in[:].rearrange("(c u) -> c u", u=1), gxl[:])
        nc.gpsimd.collective_compute("AllGather", OP.bypass,
                                     replica_groups=groups,
                                     ins=[cc0_in.opt()], outs=[cc0_out.opt()])
        c0res = pt([128, n_cores], F32, "c0res")
        nc.gpsimd.dma_start(c0res[:], cc0_out[:].rearrange("r c -> c r"))
        gxamax = pt([128, 1], F32, "gxamax")
        nc.vector.tensor_reduce(gxamax[:], c0res[:], axis=AX.X, op=OP.max)
        sx = pt([128, 1], F32, "sx")
        vts(sx[:], gxamax[:], 1.0 / 127.0, 1e-12, op0=OP.mult, op1=OP.add)
        rx = pt([128, 1], F32, "rx")
        nc.vector.reciprocal(rx[:], sx[:])

        # ---------- weights: single load, local amax, quantize ----------
        # wq[conv][g] : [128, 9*256] bf16, block k at k*256
        rw = []
        wq = []
        for ci_, wt in enumerate(wts):
            wfg = []
            wamax = pt([128, NG], F32, f"wamax{ci_}")
            for g in range(NG):
                wf = wf32.tile([128, 9 * C], F32, tag="wf32", name="wf32")
                nc.sync.dma_start(
                    wf[:].rearrange("c (k o) -> c k o", k=9),
                    wt[:, g * 128:(g + 1) * 128, :].rearrange("k c o -> c k o"))
                nc.vector.tensor_reduce(
                    wamax[:, g:g + 1], wf[:], axis=AX.X, op=OP.max,
                    apply_absolute_value=True)
                wfg.append(wf)
            wl = pt([128, 1], F32, f"wlmax{ci_}")
            nc.vector.tensor_reduce(wl[:], wamax[:], axis=AX.X, op=OP.max)
            wlt = pt([1, 128], F32, f"wlt{ci_}")
            nc.sync.dma_start(wlt[:], wl[:])
            wred = pt([1, 1], F32, f"wred{ci_}")
            nc.vector.tensor_reduce(wred[:], wlt[:], axis=AX.X, op=OP.max)
            wrep = pt([1, 128], F32, f"wrep{ci_}")
            nc.vector.tensor_scalar(wrep[:], wlt[:], wred[:, 0:1], None,
                                    op0=OP.max)
            gw = pt([128, 1], F32, f"gwmax{ci_}")
            nc.sync.dma_start(gw[:], wrep[:])
            sw = pt([128, 1], F32, f"sw{ci_}")
            vts(sw[:], gw[:], 1.0 / 127.0, 1e-12, op0=OP.mult, op1=OP.add)
            rwv = pt([128, 1], F32, f"rw{ci_}")
            nc.vector.reciprocal(rwv[:], sw[:])
            rw.append((sw, rwv))
            wqc = []
            WCH = 1152  # quantize in column chunks to keep the pool small
            for g in range(NG):
                wqg = pt([128, 9 * C], BF16, f"wq{ci_}_{g}")
                for c0_ in range(0, 9 * C, WCH):
                    wz = zrot.tile([128, WCH], F32, tag="zrot", name="zrot")
                    nc.scalar.activation(wz[:], wfg[g][:, c0_:c0_ + WCH],
                                         AF.Identity, bias=cmag[:, 0:1],
                                         scale=rwv[:, 0:1])
                    vts(wqg[:, c0_:c0_ + WCH], wz[:], -C_MAGIC, op0=OP.add)
                wqc.append(wqg)
            wq.append(wqc)

        # gamma/beta as [128, o] vectors
        gb = {}
        for name, t in (("g1", gamma1), ("b1", beta1), ("g2", gamma2), ("b2", beta2)):
            v = pt([128, NG], F32, f"gb_{name}")
            for o in range(NG):
                nc.sync.dma_start(
                    v[:, o:o + 1],
                    t[o * 128:(o + 1) * 128].rearrange("(c u) -> c u", u=1))
            gb[name] = v

        # ---------- phase B: quantize x from SBUF -> xpad (bf16) ----------
        for i in range(b_loc):
            xt, c0 = xcols(i)
            for g in range(NG):
                zx = zrot.tile([128, HW], F32, tag="zrot", name="zrot")
                nc.scalar.activation(zx[:], xt[:, c0 + g * HW:c0 + (g + 1) * HW],
                                     AF.Identity, bias=cmag[:, 0:1],
                                     scale=rx[:, 0:1])
                vts(xp3[g][i][:, 1:33, 1:33],
                    zx[:].rearrange("p (h w) -> p h w", w=32), -C_MAGIC,
                    op0=OP.add)

        # residual prefetch for c_out group 0 (sync queue drains under conv1)
        xres = [pt([128, HW], F32, f"xres{i}") for i in range(b_loc)]
        for i in range(b_loc):
            nc.sync.dma_start(xres[i][:], x_in[i, 0:128, :, :])

        # ---------- conv helper ----------
        GT = 4
        TAIL_SIZES = [4, 4, 4, 2, 1, 1]   # last group splits so only a small
        STATS_SPLIT = 12                  # payload is exposed post-conv

        def conv_group(o, wqc, post_tile, sizes=None, after_cb=None):
            pairs = [(i, s) for i in range(b_loc) for s in range(NSP)]
            if sizes is None:
                sizes = [GT] * (len(pairs) // GT)
            idx = 0
            for sz in sizes:
                grp = pairs[idx:idx + sz]
                idx += sz
                pss = [psum.tile([128, 512], F32, tag="ps", name="ps")
                       for _ in grp]
                for g in range(NG):
                    for k in range(9):
                        ky, kx = divmod(k, 3)
                        first = (g == 0) and (k == 0)
                        last = (g == NG - 1) and (k == 8)
                        wslice = wqc[g][:, k * C + o * 128: k * C + o * 128 + 128]
                        for t, (i, s) in enumerate(grp):
                            nc.tensor.matmul(
                                pss[t][:], wslice,
                                xp3[g][i][:, s * 16 + ky: s * 16 + ky + 16,
                                          kx: kx + 32],
                                start=first, stop=last)
                for t, (i, s) in enumerate(grp):
                    post_tile(i, s, i * NSP + s, pss[t])
                if after_cb is not None:
                    after_cb(idx)

        def all_gather(pay, tagp):
            """AllGather [128,S] -> SBUF view [128, S, n_cores] (strided).
            All DMAs ride the gpsimd queue (in-order with the trigger and
            never stuck behind bulk sync-queue traffic)."""
            S = pay.shape[1]
            cin = dram.tile([128, S], F32, tag=f"cg{tagp}_in", name=f"cg{tagp}_in")
            cout = dram.tile([n_cores, 128, S], F32, tag=f"cg{tagp}_out",
                             name=f"cg{tagp}_out")
            nc.gpsimd.dma_start(cin[:], pay[:])
            nc.gpsimd.collective_compute("AllGather", OP.bypass,
                                         replica_groups=groups,
                                         ins=[cin.opt()], outs=[cout.opt()])
            res = pt([128, n_cores * S], F32, f"cg{tagp}_res")
            nc.gpsimd.dma_start(
                res[:].rearrange("c (r s) -> c r s", s=S),
                cout[:].rearrange("r c s -> c r s"))
            return res.rearrange("c (r s) -> c s r", s=S)

        def bn_coeffs(gsum, s_parts, gam, bet, tag):
            """global [mean, E[x^2]] sums over cores -> A, B  (t = A*Y + B)"""
            mean = pt([128, 1], F32, f"mean{tag}")
            vts(mean[:], gsum[:, 0:1], 1.0 / n_cores, op0=OP.mult)
            e2 = pt([128, 1], F32, f"e2{tag}")
            vts(e2[:], gsum[:, 1:2], 1.0 / n_cores, op0=OP.mult)
            m2g = pt([128, 1], F32, f"m2g{tag}")
            vts(m2g[:], mean[:], mean[:, 0:1], op0=OP.mult)
            var = pt([128, 1], F32, f"var{tag}")
            nc.vector.tensor_sub(var[:], e2[:], m2g[:])
            se = pt([128, 1], F32, f"se{tag}")
            vts(se[:], s_parts[0][:], s_parts[1][:, 0:1], op0=OP.mult)
            se2 = pt([128, 1], F32, f"se2{tag}")
            vts(se2[:], se[:], se[:, 0:1], op0=OP.mult)
            se2r = pt([128, 1], F32, f"se2r{tag}")
            nc.vector.reciprocal(se2r[:], se2[:])
            epse = pt([128, 1], F32, f"epse{tag}")
            vts(epse[:], se2r[:], float(BN_EPS), op0=OP.mult)
            std = pt([128, 1], F32, f"std{tag}")
            nc.scalar.activation(std[:], var[:], AF.Sqrt, bias=epse[:, 0:1], scale=1.0)
            stdr = pt([128, 1], F32, f"stdr{tag}")
            nc.vector.reciprocal(stdr[:], std[:])
            A = pt([128, 1], F32, f"A{tag}")
            vts(A[:], gam[:], stdr[:, 0:1], op0=OP.mult)
            negmA = pt([128, 1], F32, f"negmA{tag}")
            vts(negmA[:], mean[:], A[:, 0:1], -1.0, op0=OP.mult, op1=OP.mult)
            Bv = pt([128, 1], F32, f"B{tag}")
            nc.vector.tensor_add(Bv[:], negmA[:], bet[:])
            return A, Bv

        def stat_payload(bnb, ta, tb, chmx, chmn, tagp):
            """[mean, E[x^2] (, chmax, -chmin)] over tiles [ta:tb] -> [128,S]"""
            S = 2 if chmx is None else 4
            a = pt([128, 2], F32, f"agg{tagp}")
            nc.vector.bn_aggr(a[:], bnb[:, 6 * ta: 6 * tb])
            pay = pt([128, S], F32, f"pay{tagp}")
            nc.vector.tensor_copy(pay[:, 0:1], a[:, 0:1])
            m2 = pt([128, 1], F32, f"m2{tagp}")
            vts(m2[:], a[:, 0:1], a[:, 0:1], op0=OP.mult)
            nc.vector.tensor_add(pay[:, 1:2], m2[:], a[:, 1:2])
            if chmx is not None:
                nc.vector.tensor_reduce(pay[:, 2:3], chmx[:, ta:tb],
                                        axis=AX.X, op=OP.max)
                mn = pt([128, 1], F32, f"mn{tagp}")
                nc.vector.tensor_reduce(mn[:], chmn[:, ta:tb],
                                        axis=AX.X, op=OP.min)
                vts(pay[:, 3:4], mn[:], -1.0, op0=OP.mult)
            return pay

        def combine_halves(parts, tagp):
            """weighted sum of [0:12] and [12:16] stat results + max of ranges"""
            (gvA, gvB) = parts
            S = gvA.shape[1]
            gsA = pt([128, 2], F32, f"gsA{tagp}")
            nc.vector.tensor_reduce(gsA[:], gvA[:, 0:2, :], axis=AX.X, op=OP.add)
            gsB = pt([128, 2], F32, f"gsB{tagp}")
            nc.vector.tensor_reduce(gsB[:], gvB[:, 0:2, :], axis=AX.X, op=OP.add)
            wA = pt([128, 2], F32, f"wA{tagp}")
            vts(wA[:], gsA[:], STATS_SPLIT / NT, op0=OP.mult)
            gs = pt([128, 2], F32, f"gs{tagp}")
            nc.vector.scalar_tensor_tensor(
                gs[:], gsB[:], (NT - STATS_SPLIT) / NT, wA[:],
                op0=OP.mult, op1=OP.add)
            gm = None
            if S == 4:
                gmA = pt([128, 2], F32, f"gmA{tagp}")
                nc.vector.tensor_reduce(gmA[:], gvA[:, 2:4, :], axis=AX.X, op=OP.max)
                gmB = pt([128, 2], F32, f"gmB{tagp}")
                nc.vector.tensor_reduce(gmB[:], gvB[:, 2:4, :], axis=AX.X, op=OP.max)
                gm = pt([128, 2], F32, f"gm{tagp}")
                nc.vector.tensor_max(gm[:], gmA[:], gmB[:])
            return gs, gm

        # ---------- phase C: conv1 ----------
        A1, B1, tmx = [], [], []
        for o in range(NG):
            bnb = pt([128, 6 * NT], F32, f"bnb1_{o}")
            chmx = pt([128, NT], F32, f"chmx1_{o}")
            chmn = pt([128, NT], F32, f"chmn1_{o}")

            def post1(i, s, t, ps, bnb=bnb, chmx=chmx, chmn=chmn, o=o):
                yt, yc = ycols(o, i, s)
                nc.scalar.copy(yt[:, yc:yc + 512], ps[:])
                nc.vector.bn_stats(bnb[:, 6 * t: 6 * t + 6], ps[:])
                nc.vector.tensor_reduce(chmx[:, t:t + 1], ps[:], axis=AX.X, op=OP.max)
                nc.vector.tensor_reduce(chmn[:, t:t + 1], ps[:], axis=AX.X, op=OP.min)

            if o == 0:
                conv_group(o, wq[0], post1)
                pay = stat_payload(bnb, 0, NT, chmx, chmn, f"1_{o}")
                gv = all_gather(pay, f"1_{o}")
                gs = pt([128, 2], F32, f"gs1_{o}")
                nc.vector.tensor_reduce(gs[:], gv[:, 0:2, :], axis=AX.X, op=OP.add)
                gm = pt([128, 2], F32, f"gm1_{o}")
                nc.vector.tensor_reduce(gm[:], gv[:, 2:4, :], axis=AX.X, op=OP.max)
            else:
                parts = []

                def after1(idx, bnb=bnb, chmx=chmx, chmn=chmn, parts=parts):
                    if idx == STATS_SPLIT:
                        payA = stat_payload(bnb, 0, STATS_SPLIT, chmx, chmn, "1_1a")
                        parts.append(all_gather(payA, "1_1a"))

                conv_group(o, wq[0], post1, sizes=TAIL_SIZES, after_cb=after1)
                payB = stat_payload(bnb, STATS_SPLIT, NT, chmx, chmn, "1_1b")
                parts.append(all_gather(payB, "1_1b"))
                gs, gm = combine_halves(parts, "1_1")

            a_, b_ = bn_coeffs(gs, (sx, rw[0][0]), gb["g1"][:, o:o + 1],
                               gb["b1"][:, o:o + 1], f"1_{o}")
            A1.append(a_)
            B1.append(b_)
            c1 = pt([128, 1], F32, f"c1_{o}")
            vts(c1[:], gm[:, 0:1], a_[:, 0:1], b_[:, 0:1], op0=OP.mult, op1=OP.add)
            mnv = pt([128, 1], F32, f"mnv_{o}")
            vts(mnv[:], gm[:, 1:2], -1.0, op0=OP.mult)
            c2 = pt([128, 1], F32, f"c2_{o}")
            vts(c2[:], mnv[:], a_[:, 0:1], b_[:, 0:1], op0=OP.mult, op1=OP.add)
            tm = pt([128, 1], F32, f"tmx_{o}")
            nc.vector.tensor_max(tm[:], c1[:], c2[:])
            tmx.append(tm)

        # ---------- phase D: unsigned quant scale ----------
        tmall = pt([128, 1], F32, "tmall")
        nc.vector.tensor_max(tmall[:], tmx[0][:], tmx[1][:])
        vts(tmall[:], tmall[:], 0.0, op0=OP.max)
        tgt = pt([1, 128], F32, "tgt")
        nc.sync.dma_start(tgt[:], tmall[:])
        tgr = pt([1, 1], F32, "tgr")
        nc.vector.tensor_reduce(tgr[:], tgt[:], axis=AX.X, op=OP.max)
        tgp = pt([1, 128], F32, "tgp")
        nc.vector.tensor_scalar(tgp[:], tgt[:], tgr[:, 0:1], None, op0=OP.max)
        tg = pt([128, 1], F32, "tg")
        nc.sync.dma_start(tg[:], tgp[:])
        s2q = pt([128, 1], F32, "s2q")
        vts(s2q[:], tg[:], 1.0 / 255.0, 1e-12, op0=OP.mult, op1=OP.add)
        r2q = pt([128, 1], F32, "r2q")
        nc.vector.reciprocal(r2q[:], s2q[:])
        A1p, B1C = [], []
        for o in range(NG):
            ap_ = pt([128, 1], F32, f"A1p_{o}")
            vts(ap_[:], A1[o][:], r2q[:, 0:1], op0=OP.mult)
            bp_ = pt([128, 1], F32, f"B1C_{o}")
            vts(bp_[:], B1[o][:], r2q[:, 0:1], C_MAGIC, op0=OP.mult, op1=OP.add)
            A1p.append(ap_)
            B1C.append(bp_)

        # ---------- phase E: quantize Y1 (SBUF) -> xpad ----------
        # q = relu(round(A1p*Y + B1p)): one fused ACT (scale, bias incl +C)
        # then one DVE op (-C with relu) straight into the bf16 xpad interior.
        for i in range(b_loc):
            for g in range(NG):
                yt, yc = ycols(g, i, 0)
                z1 = zrot.tile([128, HW], F32, tag="zrot", name="zrot")
                nc.scalar.activation(z1[:], yt[:, yc:yc + HW], AF.Identity,
                                     bias=B1C[g][:, 0:1], scale=A1p[g][:, 0:1])
                nc.vector.tensor_scalar(
                    xp3[g][i][:, 1:33, 1:33],
                    z1[:].rearrange("p (h w) -> p h w", w=32),
                    -C_MAGIC, 0.0, op0=OP.add, op1=OP.max)

        # ---------- phase F/G/H: conv2 per group + BN2 + final epilogue ------
        for o in range(NG):
            bnb = pt([128, 6 * NT], F32, f"bnb2_{o}")

            def post2(i, s, t, ps, bnb=bnb, o=o):
                yt, yc = ycols(o, i, s)
                nc.scalar.copy(yt[:, yc:yc + 512], ps[:])
                nc.vector.bn_stats(bnb[:, 6 * t: 6 * t + 6], ps[:])

            if o == 0:
                conv_group(o, wq[1], post2)
                pay = stat_payload(bnb, 0, NT, None, None, f"2_{o}")
                gv = all_gather(pay, f"2_{o}")
                gs2 = pt([128, 2], F32, f"gs2_{o}")
                nc.vector.tensor_reduce(gs2[:], gv[:, 0:2, :], axis=AX.X, op=OP.add)
            else:
                # residual prefetch for group 1 reuses the xres tiles (WAR
                # with the group-0 epilogue reads; loads drain under conv2)
                for i in range(b_loc):
                    nc.sync.dma_start(xres[i][:], x_in[i, 128:256, :, :])
                parts = []

                def after2(idx, bnb=bnb, parts=parts):
                    if idx == STATS_SPLIT:
                        payA = stat_payload(bnb, 0, STATS_SPLIT, None, None, "2_1a")
                        parts.append(all_gather(payA, "2_1a"))

                conv_group(o, wq[1], post2, sizes=TAIL_SIZES, after_cb=after2)
                payB = stat_payload(bnb, STATS_SPLIT, NT, None, None, "2_1b")
                parts.append(all_gather(payB, "2_1b"))
                gs2, _ = combine_halves(parts, "2_1")

            A2, B2 = bn_coeffs(gs2, (s2q, rw[1][0]), gb["g2"][:, o:o + 1],
                               gb["b2"][:, o:o + 1], f"2_{o}")
            # final: relu(A2*Y2 + B2 + x), one [128,1024] tile per image
            for i in range(b_loc):
                yt, yc = ycols(o, i, 0)
                tt = trot.tile([128, HW], F32, tag="trot", name="trot")
                nc.vector.scalar_tensor_tensor(
                    tt[:], yt[:, yc:yc + HW], A2[:, 0:1],
                    xres[i][:], op0=OP.mult, op1=OP.add)
                osb = orot.tile([128, HW], F32, tag="orot", name="orot")
                nc.scalar.activation(osb[:], tt[:], AF.Relu,
                                     bias=B2[:, 0:1], scale=1.0)
                nc.sync.dma_start(out[i, o * 128:(o + 1) * 128, :, :], osb[:])

    nc.compile()
    _NC_CACHE[key] = nc
    return nc


def _prep_host(x, w1, w2, gamma1, beta1, gamma2, beta2, n_cores):
    w1t = np.ascontiguousarray(
        np.transpose(np.asarray(w1, np.float32), (2, 3, 1, 0)).reshape(9, C, C))
    w2t = np.ascontiguousarray(
        np.transpose(np.asarray(w2, np.float32), (2, 3, 1, 0)).reshape(9, C, C))
    x = np.ascontiguousarray(np.asarray(x, np.float32))
    b_loc = x.shape[0] // n_cores
    in_maps = []
    for c in range(n_cores):
        in_maps.append({
            "x": x[c * b_loc:(c + 1) * b_loc],
            "w1t": w1t, "w2t": w2t,
            "gamma1": np.asarray(gamma1, np.float32),
            "beta1": np.asarray(beta1, np.float32),
            "gamma2": np.asarray(gamma2, np.float32),
            "beta2": np.asarray(beta2, np.float32),
        })
    return in_maps, b_loc


def kernel(x, w1, gamma1, beta1, w2, gamma2, beta2, _trace=False):
    in_maps, b_loc = _prep_host(x, w1, w2, gamma1, beta1, gamma2, beta2, N_CORES)
    nc = build_nc(b_loc, N_CORES)
    res = run_bass_kernel_spmd(nc, in_maps, list(range(N_CORES)), trace=_trace)
    out = np.concatenate(
        [np.asarray(res.results[c]["out"]).reshape(b_loc, C, H, W)
         for c in range(N_CORES)], axis=0)
    if _trace:
        kernel._last_results = res
    return out


# revision 18
# speedup vs baseline: 1.0824x; 1.0137x over previous
"""Trainium2 Bass kernel for a quantized ResNet BasicBlock (dense_cnn).

  y = relu(bn2(conv2(uq(relu(bn1(conv1(q(x), q(w1)))))), q(w2)) + x)

Strategy (8 NeuronCores, data-parallel over batch):
  - Each core processes B_LOC = B/8 images; conv weights + BN params replicated.
  - Per-tensor symmetric quantization produces small integers held in bf16
    (ints <= 256 exact); 3x3 convs run as 9 shifted matmuls per c_in group
    accumulating in fp32 PSUM => exact integer arithmetic.
  - Quant scales factor out of batchnorm; all BN math in the integer domain.
  - v2 layout: x is loaded ONCE into SBUF (XY tiles) and never re-read for the
    quantize pass; conv1 output Y1 lives in the same SBUF tiles (x is dead
    after quantize), conv2 output Y2 again reuses them.  The residual is
    prefetched into SBUF during the convs.  No DRAM spill/reload of Y1.
  - Collectives: warmup AllReduce triggered at t~0 (absorbs comm-channel
    init), AllGather everywhere else (shorter mesh than AllReduce); per-group
    stats split [0:12]/[12:16] with conv tail groups [2,1,1] so only a 4-tile
    payload is exposed after the last matmul; collective payload DMAs ride
    the gpsimd queue (never stuck behind bulk traffic on the sync queue).
  - Rounding replicates round-to-nearest-even via the +/- 1.5*2^23 trick.
"""

import numpy as np
from contextlib import ExitStack

import concourse.bass as bass
import concourse.mybir as mybir
import concourse.tile as tile
import concourse.bass_isa as bass_isa
from concourse import bacc
from concourse.bass_utils import run_bass_kernel_spmd

F32 = mybir.dt.float32
BF16 = mybir.dt.bfloat16
AF = mybir.ActivationFunctionType
OP = mybir.AluOpType
AX = mybir.AxisListType

C_MAGIC = 12582912.0  # 1.5 * 2^23 : fp32 add/sub rounds to nearest-even integer
BN_EPS = 1e-5

N_CORES = 8
B = 64          # full batch
C = 256         # channels
H = W = 32
HW = H * W      # 1024
NG = 2          # channel groups of 128
NSP = 2         # spatial halves (16 rows x 32 cols = 512) per image
PHW_ = 34 * 34  # padded image size

_NC_CACHE = {}


def build_nc(b_loc=B // N_CORES, n_cores=N_CORES):
    key = (b_loc, n_cores)
    if key in _NC_CACHE:
        return _NC_CACHE[key]

    nc = bacc.Bacc("TRN2", target_bir_lowering=False, debug=False,
                   num_devices=n_cores)
    groups = [list(range(n_cores))]

    x_in = nc.dram_tensor("x", [b_loc, C, H, W], F32, kind="ExternalInput").ap()
    w1t = nc.dram_tensor("w1t", [9, C, C], F32, kind="ExternalInput").ap()
    w2t = nc.dram_tensor("w2t", [9, C, C], F32, kind="ExternalInput").ap()
    gamma1 = nc.dram_tensor("gamma1", [C], F32, kind="ExternalInput").ap()
    beta1 = nc.dram_tensor("beta1", [C], F32, kind="ExternalInput").ap()
    gamma2 = nc.dram_tensor("gamma2", [C], F32, kind="ExternalInput").ap()
    beta2 = nc.dram_tensor("beta2", [C], F32, kind="ExternalInput").ap()
    out = nc.dram_tensor("out", [b_loc, C, H, W], F32, kind="ExternalOutput").ap()

    wts = [w1t, w2t]
    NT = b_loc * NSP          # psum tiles per c_out group per conv (16)

    with tile.TileContext(nc) as tc, ExitStack() as ctx:
        per = ctx.enter_context(tc.tile_pool(name="persist", bufs=1))
        wf32 = ctx.enter_context(tc.tile_pool(name="wf32", bufs=2))
        zrot = ctx.enter_context(tc.tile_pool(name="zrot", bufs=2))
        orot = ctx.enter_context(tc.tile_pool(name="orot", bufs=2))
        trot = ctx.enter_context(tc.tile_pool(name="trot", bufs=2))
        psum = ctx.enter_context(tc.tile_pool(name="psum", bufs=8, space="PSUM"))
        dram = ctx.enter_context(tc.tile_pool(name="dram", bufs=1, space="DRAM"))

        def pt(shape, dtype, name):
            return per.tile(shape, dtype, tag=name, name=name)

        def vts(outap, inap, s1, s2=None, op0=OP.mult, op1=None):
            if op1 is None:
                nc.vector.tensor_scalar(outap, inap, s1, None, op0=op0)
            else:
                nc.vector.tensor_scalar(outap, inap, s1, s2, op0=op0, op1=op1)

        def mk_epse(s_act, s_w, tag):
            """eps / (s_act*s_w)^2 - precomputed off the critical path"""
            se = pt([128, 1], F32, f"se{tag}")
            vts(se[:], s_act[:], s_w[:, 0:1], op0=OP.mult)
            se2 = pt([128, 1], F32, f"se2{tag}")
            vts(se2[:], se[:], se[:, 0:1], op0=OP.mult)
            se2r = pt([128, 1], F32, f"se2r{tag}")
            nc.vector.reciprocal(se2r[:], se2[:])
            epse = pt([128, 1], F32, f"epse{tag}")
            vts(epse[:], se2r[:], float(BN_EPS), op0=OP.mult)
            return epse

        # ---------- warmup collective: very first gpsimd instruction --------
        # AllGather (2 mesh phases, vs 4 for AllReduce) on an uninitialized
        # DRAM tile (pure data movement, result unused) - no producer means
        # no semaphore wait, so the trigger fires the moment the gpsimd
        # engine starts (~10us) and the ~55us comm-channel init + peer-skew
        # wait overlaps the x load / amax / weight-quantize phase.
        wu_in = dram.tile([128], F32, tag="wu_in", name="wu_in")
        wu_out = dram.tile([n_cores, 128], F32, tag="wu_out", name="wu_out")
        nc.gpsimd.collective_compute("AllGather", OP.bypass, replica_groups=groups,
                                     ins=[wu_in.opt()], outs=[wu_out.opt()])

        # padded quantized input tiles; memset on DVE (gpsimd queue must stay
        # free so the warmup collective triggers at t~0)
        xpad = [[None] * b_loc for _ in range(NG)]
        xp3 = [[None] * b_loc for _ in range(NG)]
        for g in range(NG):
            for i in range(b_loc):
                t = pt([128, PHW_], BF16, f"xpad{g}_{i}")
                nc.vector.memset(t[:], 0.0)
                xpad[g][i] = t
                xp3[g][i] = t.rearrange("p (h w) -> p h w", w=34)

        # ---------- constants ----------
        cmag = pt([128, 1], F32, "cmag")
        nc.vector.memset(cmag[:], C_MAGIC)

        # ---------- phase A: x -> SBUF (kept!), local amax -> C0 AllGather --
        XY = [pt([128, (b_loc // 2) * 2048], F32, f"XY{h}") for h in range(2)]

        def xcols(i):      # x image i lives in XY[i//4] cols (i%4)*2048
            return XY[i // (b_loc // 2)], (i % (b_loc // 2)) * 2048

        def ycols(o, i, s):  # conv output (o,i,s) -> XY[o] cols i*1024+s*512
            return XY[o], i * 1024 + s * 512

        xamax = pt([128, b_loc], F32, "xamax")
        for i in range(b_loc):
            xt, c0 = xcols(i)
            nc.sync.dma_start(
                xt[:, c0:c0 + 2048].rearrange("c (g hw) -> c g hw", g=NG),
                x_in[i].rearrange("(g c) h w -> c g (h w)", c=128))
            nc.vector.tensor_reduce(
                xamax[:, i:i + 1],
                xt[:, c0:c0 + 2048].rearrange("c (g hw) -> c g hw", g=NG),
                axis=AX.XY, op=OP.max, apply_absolute_value=True)
        lamax = pt([128, 1], F32, "lamax")
        nc.vector.tensor_reduce(lamax[:], xamax[:], axis=AX.X, op=OP.max)
        # cross-partition max BEFORE the collective (in slack time): the
        # C0 readback then needs no transpose hop on the critical path.
        lat = pt([1, 128], F32, "lat")
        nc.sync.dma_start(lat[:], lamax[:])
        lar = pt([1, 1], F32, "lar")
        nc.vector.tensor_reduce(lar[:], lat[:], axis=AX.X, op=OP.max)
        lap = pt([1, 128], F32, "lap")
        nc.vector.tensor_scalar(lap[:], lat[:], lar[:, 0:1], None, op0=OP.max)
        gxl = pt([128, 1], F32, "gxl")
        nc.sync.dma_start(gxl[:], lap[:])
        cc0_in = dram.tile([128], F32, tag="cc0_in", name="cc0_in")
        cc0_out = dram.tile([n_cores, 128], F32, tag="cc0_out", name="cc0_out")
        nc.gpsimd.dma_start(cc0_in[:].rearrange("(c u) -> c u", u=1), gxl[:])
        nc.gpsimd.collective_compute("AllGather", OP.bypass,
                                     replica_groups=groups,
                                     ins=[cc0_in.opt()], outs=[cc0_out.opt()])
        c0res = pt([128, n_cores], F32, "c0res")
        nc.gpsimd.dma_start(c0res[:], cc0_out[:].rearrange("r c -> c r"))
        gxamax = pt([128, 1], F32, "gxamax")
        nc.vector.tensor_reduce(gxamax[:], c0res[:], axis=AX.X, op=OP.max)
        sx = pt([128, 1], F32, "sx")
        vts(sx[:], gxamax[:], 1.0 / 127.0, 1e-12, op0=OP.mult, op1=OP.add)
        rx = pt([128, 1], F32, "rx")
        nc.vector.reciprocal(rx[:], sx[:])

        # ---------- weights: single load, local amax, quantize ----------
        # wq[conv][g] : [128, 9*256] bf16, block k at k*256
        rw = []
        wq = []
        for ci_, wt in enumerate(wts):
            wfg = []
            wamax = pt([128, NG], F32, f"wamax{ci_}")
            for g in range(NG):
                wf = wf32.tile([128, 9 * C], F32, tag="wf32", name="wf32")
                nc.sync.dma_start(
                    wf[:].rearrange("c (k o) -> c k o", k=9),
                    wt[:, g * 128:(g + 1) * 128, :].rearrange("k c o -> c k o"))
                nc.vector.tensor_reduce(
                    wamax[:, g:g + 1], wf[:], axis=AX.X, op=OP.max,
                    apply_absolute_value=True)
                wfg.append(wf)
            wl = pt([128, 1], F32, f"wlmax{ci_}")
            nc.vector.tensor_reduce(wl[:], wamax[:], axis=AX.X, op=OP.max)
            wlt = pt([1, 128], F32, f"wlt{ci_}")
            nc.sync.dma_start(wlt[:], wl[:])
            wred = pt([1, 1], F32, f"wred{ci_}")
            nc.vector.tensor_reduce(wred[:], wlt[:], axis=AX.X, op=OP.max)
            wrep = pt([1, 128], F32, f"wrep{ci_}")
            nc.vector.tensor_scalar(wrep[:], wlt[:], wred[:, 0:1], None,
                                    op0=OP.max)
            gw = pt([128, 1], F32, f"gwmax{ci_}")
            nc.sync.dma_start(gw[:], wrep[:])
            sw = pt([128, 1], F32, f"sw{ci_}")
            vts(sw[:], gw[:], 1.0 / 127.0, 1e-12, op0=OP.mult, op1=OP.add)
            rwv = pt([128, 1], F32, f"rw{ci_}")
            nc.vector.reciprocal(rwv[:], sw[:])
            rw.append((sw, rwv))
            wqc = []
            WCH = 1152  # quantize in column chunks to keep the pool small
            for g in range(NG):
                wqg = pt([128, 9 * C], BF16, f"wq{ci_}_{g}")
                for c0_ in range(0, 9 * C, WCH):
                    wz = zrot.tile([128, WCH], F32, tag="zrot", name="zrot")
                    nc.scalar.activation(wz[:], wfg[g][:, c0_:c0_ + WCH],
                                         AF.Identity, bias=cmag[:, 0:1],
                                         scale=rwv[:, 0:1])
                    vts(wqg[:, c0_:c0_ + WCH], wz[:], -C_MAGIC, op0=OP.add)
                wqc.append(wqg)
            wq.append(wqc)

        # gamma/beta as [128, o] vectors
        gb = {}
        for name, t in (("g1", gamma1), ("b1", beta1), ("g2", gamma2), ("b2", beta2)):
            v = pt([128, NG], F32, f"gb_{name}")
            for o in range(NG):
                nc.sync.dma_start(
                    v[:, o:o + 1],
                    t[o * 128:(o + 1) * 128].rearrange("(c u) -> c u", u=1))
            gb[name] = v

        epse1 = mk_epse(sx, rw[0][0], "1")  # runs pre-conv, off critical path

        # ---------- phase B: quantize x from SBUF -> xpad (bf16) ----------
        # first two images gate conv1's first matmuls: group 1's scale+bias
        # runs on DVE so ACT and DVE work in parallel.
        for i in range(b_loc):
            xt, c0 = xcols(i)
            for g in range(NG):
                zx = zrot.tile([128, HW], F32, tag="zrot", name="zrot")
                if i < 2 and g == 1:
                    nc.vector.tensor_scalar(zx[:],
                                            xt[:, c0 + g * HW:c0 + (g + 1) * HW],
                                            rx[:, 0:1], cmag[:, 0:1],
                                            op0=OP.mult, op1=OP.add)
                else:
                    nc.scalar.activation(zx[:],
                                         xt[:, c0 + g * HW:c0 + (g + 1) * HW],
                                         AF.Identity, bias=cmag[:, 0:1],
                                         scale=rx[:, 0:1])
                vts(xp3[g][i][:, 1:33, 1:33],
                    zx[:].rearrange("p (h w) -> p h w", w=32), -C_MAGIC,
                    op0=OP.add)

        # residual prefetch for c_out group 0 (sync queue drains under conv1)
        xres = [pt([128, HW], F32, f"xres{i}") for i in range(b_loc)]
        for i in range(b_loc):
            nc.sync.dma_start(xres[i][:], x_in[i, 0:128, :, :])

        # ---------- conv helper ----------
        GT = 4
        TAIL_SIZES = [4, 4, 4, 2, 1, 1]   # last group splits so only a small
        STATS_SPLIT = 12                  # payload is exposed post-conv

        def conv_group(o, wqc, post_tile, sizes=None, after_cb=None):
            pairs = [(i, s) for i in range(b_loc) for s in range(NSP)]
            if sizes is None:
                sizes = [GT] * (len(pairs) // GT)
            idx = 0
            for sz in sizes:
                grp = pairs[idx:idx + sz]
                idx += sz
                pss = [psum.tile([128, 512], F32, tag="ps", name="ps")
                       for _ in grp]
                for g in range(NG):
                    for k in range(9):
                        ky, kx = divmod(k, 3)
                        first = (g == 0) and (k == 0)
                        last = (g == NG - 1) and (k == 8)
                        wslice = wqc[g][:, k * C + o * 128: k * C + o * 128 + 128]
                        for t, (i, s) in enumerate(grp):
                            nc.tensor.matmul(
                                pss[t][:], wslice,
                                xp3[g][i][:, s * 16 + ky: s * 16 + ky + 16,
                                          kx: kx + 32],
                                start=first, stop=last)
                for t, (i, s) in enumerate(grp):
                    post_tile(i, s, i * NSP + s, pss[t])
                if after_cb is not None:
                    after_cb(idx)

        def all_gather(pay, tagp):
            """AllGather [128,S] -> SBUF view [128, S, n_cores] (strided).
            All DMAs ride the gpsimd queue (in-order with the trigger and
            never stuck behind bulk sync-queue traffic)."""
            S = pay.shape[1]
            cin = dram.tile([128, S], F32, tag=f"cg{tagp}_in", name=f"cg{tagp}_in")
            cout = dram.tile([n_cores, 128, S], F32, tag=f"cg{tagp}_out",
                             name=f"cg{tagp}_out")
            nc.gpsimd.dma_start(cin[:], pay[:])
            nc.gpsimd.collective_compute("AllGather", OP.bypass,
                                         replica_groups=groups,
                                         ins=[cin.opt()], outs=[cout.opt()])
            res = pt([128, n_cores * S], F32, f"cg{tagp}_res")
            nc.gpsimd.dma_start(
                res[:].rearrange("c (r s) -> c r s", s=S),
                cout[:].rearrange("r c s -> c r s"))
            return res.rearrange("c (r s) -> c s r", s=S)

        def bn_coeffs(gsum, epse, gam, bet, tag):
            """global [mean, E[x^2]] sums over cores -> A, B  (t = A*Y + B);
            minimal-op chain (7 ops) since it sits on the critical path."""
            me2 = pt([128, 2], F32, f"me2{tag}")
            vts(me2[:], gsum[:], 1.0 / n_cores, op0=OP.mult)
            negvar = pt([128, 1], F32, f"negvar{tag}")
            nc.vector.scalar_tensor_tensor(
                negvar[:], me2[:, 0:1], me2[:, 0:1], me2[:, 1:2],
                op0=OP.mult, op1=OP.subtract)
            std = pt([128, 1], F32, f"std{tag}")
            nc.scalar.activation(std[:], negvar[:], AF.Sqrt,
                                 bias=epse[:, 0:1], scale=-1.0)
            stdr = pt([128, 1], F32, f"stdr{tag}")
            nc.vector.reciprocal(stdr[:], std[:])
            A = pt([128, 1], F32, f"A{tag}")
            vts(A[:], gam[:], stdr[:, 0:1], op0=OP.mult)
            negmA = pt([128, 1], F32, f"negmA{tag}")
            vts(negmA[:], me2[:, 0:1], A[:, 0:1], -1.0, op0=OP.mult, op1=OP.mult)
            Bv = pt([128, 1], F32, f"B{tag}")
            nc.vector.tensor_add(Bv[:], negmA[:], bet[:])
            return A, Bv

        def stat_payload(pay, pc, bnb, ta, tb, chmx, chmn, tagp):
            """write [mean, E[x^2] (, chmax, -chmin)] over tiles [ta:tb] into
            pay[:, pc:pc+S]"""
            a = pt([128, 2], F32, f"agg{tagp}")
            nc.vector.bn_aggr(a[:], bnb[:, 6 * ta: 6 * tb])
            nc.vector.tensor_copy(pay[:, pc:pc + 1], a[:, 0:1])
            m2 = pt([128, 1], F32, f"m2{tagp}")
            vts(m2[:], a[:, 0:1], a[:, 0:1], op0=OP.mult)
            nc.vector.tensor_add(pay[:, pc + 1:pc + 2], m2[:], a[:, 1:2])
            if chmx is not None:
                nc.vector.tensor_reduce(pay[:, pc + 2:pc + 3], chmx[:, ta:tb],
                                        axis=AX.X, op=OP.max)
                mn = pt([128, 1], F32, f"mn{tagp}")
                nc.vector.tensor_reduce(mn[:], chmn[:, ta:tb],
                                        axis=AX.X, op=OP.min)
                vts(pay[:, pc + 3:pc + 4], mn[:], -1.0, op0=OP.mult)

        def combine_halves(gvA, cA, gvB, cB, with_mx, tagp):
            """weighted sum of [0:12] and [12:16] stat results + max of ranges.
            gvA/gvB are [128, S, n_cores] AG views; cA/cB column offsets."""
            gsA = pt([128, 2], F32, f"gsA{tagp}")
            nc.vector.tensor_reduce(gsA[:], gvA[:, cA:cA + 2, :], axis=AX.X, op=OP.add)
            gsB = pt([128, 2], F32, f"gsB{tagp}")
            nc.vector.tensor_reduce(gsB[:], gvB[:, cB:cB + 2, :], axis=AX.X, op=OP.add)
            wA = pt([128, 2], F32, f"wA{tagp}")
            vts(wA[:], gsA[:], STATS_SPLIT / NT, op0=OP.mult)
            gs = pt([128, 2], F32, f"gs{tagp}")
            nc.vector.scalar_tensor_tensor(
                gs[:], gsB[:], (NT - STATS_SPLIT) / NT, wA[:],
                op0=OP.mult, op1=OP.add)
            gm = None
            if with_mx:
                gmA = pt([128, 2], F32, f"gmA{tagp}")
                nc.vector.tensor_reduce(gmA[:], gvA[:, cA + 2:cA + 4, :],
                                        axis=AX.X, op=OP.max)
                gmB = pt([128, 2], F32, f"gmB{tagp}")
                nc.vector.tensor_reduce(gmB[:], gvB[:, cB + 2:cB + 4, :],
                                        axis=AX.X, op=OP.max)
                gm = pt([128, 2], F32, f"gm{tagp}")
                nc.vector.tensor_max(gm[:], gmA[:], gmB[:])
            return gs, gm

        # ---------- phase C: conv1 ----------
        # Stats collectives: o=0's full-group stats ride the same AllGather
        # as o=1's [0:12] partial (one merged [128,8] payload, fired 12/16
        # through conv1-o1); only the small [12:16] payload is exposed after
        # the last matmul.
        bnb1 = [pt([128, 6 * NT], F32, f"bnb1_{o}") for o in range(NG)]
        chmx1 = [pt([128, NT], F32, f"chmx1_{o}") for o in range(NG)]
        chmn1 = [pt([128, NT], F32, f"chmn1_{o}") for o in range(NG)]
        pay1m = pt([128, 8], F32, "pay1m")
        pay1b = pt([128, 4], F32, "pay1b")

        def mk_post1(o):
            bnb, chmx, chmn = bnb1[o], chmx1[o], chmn1[o]

            def post1(i, s, t, ps):
                yt, yc = ycols(o, i, s)
                nc.scalar.copy(yt[:, yc:yc + 512], ps[:])
                nc.vector.bn_stats(bnb[:, 6 * t: 6 * t + 6], ps[:])
                nc.vector.tensor_reduce(chmx[:, t:t + 1], ps[:], axis=AX.X, op=OP.max)
                nc.vector.tensor_reduce(chmn[:, t:t + 1], ps[:], axis=AX.X, op=OP.min)
            return post1

        conv_group(0, wq[0], mk_post1(0))
        stat_payload(pay1m, 0, bnb1[0], 0, NT, chmx1[0], chmn1[0], "1_0")
        gv1m_box = []

        def after1(idx):
            if idx == STATS_SPLIT:
                stat_payload(pay1m, 4, bnb1[1], 0, STATS_SPLIT,
                             chmx1[1], chmn1[1], "1_1a")
                gv1m_box.append(all_gather(pay1m, "1m"))

        conv_group(1, wq[0], mk_post1(1), sizes=TAIL_SIZES, after_cb=after1)
        stat_payload(pay1b, 0, bnb1[1], STATS_SPLIT, NT, chmx1[1], chmn1[1], "1_1b")
        gv1b = all_gather(pay1b, "1b")
        gv1m = gv1m_box[0]

        A1, B1, tmx = [], [], []
        for o in range(NG):
            if o == 0:
                gs = pt([128, 2], F32, "gs1_0")
                nc.vector.tensor_reduce(gs[:], gv1m[:, 0:2, :], axis=AX.X, op=OP.add)
                gm = pt([128, 2], F32, "gm1_0")
                nc.vector.tensor_reduce(gm[:], gv1m[:, 2:4, :], axis=AX.X, op=OP.max)
            else:
                gs, gm = combine_halves(gv1m, 4, gv1b, 0, True, "1_1")
            a_, b_ = bn_coeffs(gs, epse1, gb["g1"][:, o:o + 1],
                               gb["b1"][:, o:o + 1], f"1_{o}")
            A1.append(a_)
            B1.append(b_)
            c1 = pt([128, 1], F32, f"c1_{o}")
            vts(c1[:], gm[:, 0:1], a_[:, 0:1], b_[:, 0:1], op0=OP.mult, op1=OP.add)
            mnv = pt([128, 1], F32, f"mnv_{o}")
            vts(mnv[:], gm[:, 1:2], -1.0, op0=OP.mult)
            c2 = pt([128, 1], F32, f"c2_{o}")
            vts(c2[:], mnv[:], a_[:, 0:1], b_[:, 0:1], op0=OP.mult, op1=OP.add)
            tm = pt([128, 1], F32, f"tmx_{o}")
            nc.vector.tensor_max(tm[:], c1[:], c2[:])
            tmx.append(tm)

        # ---------- phase D: unsigned quant scale ----------
        # (relu clamp folded into the tgp broadcast; cross-partition max via
        # the DMA-transpose trick)
        tmall = pt([128, 1], F32, "tmall")
        nc.vector.tensor_max(tmall[:], tmx[0][:], tmx[1][:])
        tgt = pt([1, 128], F32, "tgt")
        nc.sync.dma_start(tgt[:], tmall[:])
        tgr = pt([1, 1], F32, "tgr")
        nc.vector.tensor_reduce(tgr[:], tgt[:], axis=AX.X, op=OP.max)
        tgp = pt([1, 128], F32, "tgp")
        nc.vector.tensor_scalar(tgp[:], tgt[:], tgr[:, 0:1], 0.0,
                                op0=OP.max, op1=OP.max)
        tg = pt([128, 1], F32, "tg")
        nc.sync.dma_start(tg[:], tgp[:])
        s2q = pt([128, 1], F32, "s2q")
        vts(s2q[:], tg[:], 1.0 / 255.0, 1e-12, op0=OP.mult, op1=OP.add)
        r2q = pt([128, 1], F32, "r2q")
        nc.vector.reciprocal(r2q[:], s2q[:])
        A1p, B1C = [], []
        for o in range(NG):
            ap_ = pt([128, 1], F32, f"A1p_{o}")
            vts(ap_[:], A1[o][:], r2q[:, 0:1], op0=OP.mult)
            bp_ = pt([128, 1], F32, f"B1C_{o}")
            vts(bp_[:], B1[o][:], r2q[:, 0:1], C_MAGIC, op0=OP.mult, op1=OP.add)
            A1p.append(ap_)
            B1C.append(bp_)

        # ---------- phase E: quantize Y1 (SBUF) -> xpad ----------
        # q = relu(round(A1p*Y + B1p)): one fused scale+bias(+C) op, then one
        # DVE op (-C with relu) straight into the bf16 xpad interior.  For
        # the first two images (which gate conv2's first matmuls) group 1's
        # scale+bias runs on DVE so ACT and DVE work in parallel.
        for i in range(b_loc):
            for g in range(NG):
                yt, yc = ycols(g, i, 0)
                z1 = zrot.tile([128, HW], F32, tag="zrot", name="zrot")
                if i < 2 and g == 1:
                    nc.vector.tensor_scalar(z1[:], yt[:, yc:yc + HW],
                                            A1p[g][:, 0:1], B1C[g][:, 0:1],
                                            op0=OP.mult, op1=OP.add)
                else:
                    nc.scalar.activation(z1[:], yt[:, yc:yc + HW], AF.Identity,
                                         bias=B1C[g][:, 0:1], scale=A1p[g][:, 0:1])
                nc.vector.tensor_scalar(
                    xp3[g][i][:, 1:33, 1:33],
                    z1[:].rearrange("p (h w) -> p h w", w=32),
                    -C_MAGIC, 0.0, op0=OP.add, op1=OP.max)

        epse2 = mk_epse(s2q, rw[1][0], "2")  # hidden under conv2

        # ---------- phase F/G/H: conv2 per group + BN2 + final epilogue ------
        for o in range(NG):
            bnb = pt([128, 6 * NT], F32, f"bnb2_{o}")

            def post2(i, s, t, ps, bnb=bnb, o=o):
                yt, yc = ycols(o, i, s)
                nc.scalar.copy(yt[:, yc:yc + 512], ps[:])
                nc.vector.bn_stats(bnb[:, 6 * t: 6 * t + 6], ps[:])

            if o == 0:
                conv_group(o, wq[1], post2)
                pay = pt([128, 2], F32, "pay2_0")
                stat_payload(pay, 0, bnb, 0, NT, None, None, "2_0")
                gv = all_gather(pay, "2_0")
                gs2 = pt([128, 2], F32, f"gs2_{o}")
                nc.vector.tensor_reduce(gs2[:], gv[:, 0:2, :], axis=AX.X, op=OP.add)
            else:
                # residual prefetch for group 1 reuses the xres tiles (WAR
                # with the group-0 epilogue reads; loads drain under conv2)
                for i in range(b_loc):
                    nc.sync.dma_start(xres[i][:], x_in[i, 128:256, :, :])
                payA = pt([128, 2], F32, "pay2_1a")
                payB = pt([128, 2], F32, "pay2_1b")
                parts = []

                def after2(idx, bnb=bnb, parts=parts):
                    if idx == STATS_SPLIT:
                        stat_payload(payA, 0, bnb, 0, STATS_SPLIT, None, None, "2_1a")
                        parts.append(all_gather(payA, "2_1a"))

                conv_group(o, wq[1], post2, sizes=TAIL_SIZES, after_cb=after2)
                stat_payload(payB, 0, bnb, STATS_SPLIT, NT, None, None, "2_1b")
                gvB = all_gather(payB, "2_1b")
                gs2, _ = combine_halves(parts[0], 0, gvB, 0, False, "2_1")

            A2, B2 = bn_coeffs(gs2, epse2, gb["g2"][:, o:o + 1],
                               gb["b2"][:, o:o + 1], f"2_{o}")
            # final: relu(A2*Y2 + B2 + x), one [128,1024] tile per image
            for i in range(b_loc):
                yt, yc = ycols(o, i, 0)
                tt = trot.tile([128, HW], F32, tag="trot", name="trot")
                nc.vector.scalar_tensor_tensor(
                    tt[:], yt[:, yc:yc + HW], A2[:, 0:1],
                    xres[i][:], op0=OP.mult, op1=OP.add)
                osb = orot.tile([128, HW], F32, tag="orot", name="orot")
                nc.scalar.activation(osb[:], tt[:], AF.Relu,
                                     bias=B2[:, 0:1], scale=1.0)
                nc.sync.dma_start(out[i, o * 128:(o + 1) * 128, :, :], osb[:])

    nc.compile()
    _NC_CACHE[key] = nc
    return nc


def _prep_host(x, w1, w2, gamma1, beta1, gamma2, beta2, n_cores):
    w1t = np.ascontiguousarray(
        np.transpose(np.asarray(w1, np.float32), (2, 3, 1, 0)).reshape(9, C, C))
    w2t = np.ascontiguousarray(
        np.transpose(np.asarray(w2, np.float32), (2, 3, 1, 0)).reshape(9, C, C))
    x = np.ascontiguousarray(np.asarray(x, np.float32))
    b_loc = x.shape[0] // n_cores
    in_maps = []
    for c in range(n_cores):
        in_maps.append({
            "x": x[c * b_loc:(c + 1) * b_loc],
            "w1t": w1t, "w2t": w2t,
            "gamma1": np.asarray(gamma1, np.float32),
            "beta1": np.asarray(beta1, np.float32),
            "gamma2": np.asarray(gamma2, np.float32),
            "beta2": np.asarray(beta2, np.float32),
        })
    return in_maps, b_loc


def kernel(x, w1, gamma1, beta1, w2, gamma2, beta2, _trace=False):
    in_maps, b_loc = _prep_host(x, w1, w2, gamma1, beta1, gamma2, beta2, N_CORES)
    nc = build_nc(b_loc, N_CORES)
    res = run_bass_kernel_spmd(nc, in_maps, list(range(N_CORES)), trace=_trace)
    out = np.concatenate(
        [np.asarray(res.results[c]["out"]).reshape(b_loc, C, H, W)
         for c in range(N_CORES)], axis=0)
    if _trace:
        kernel._last_results = res
    return out


# revision 24
# speedup vs baseline: 1.0862x; 1.0034x over previous
"""Trainium2 Bass kernel for a quantized ResNet BasicBlock (dense_cnn).

  y = relu(bn2(conv2(uq(relu(bn1(conv1(q(x), q(w1)))))), q(w2)) + x)

Strategy (8 NeuronCores, data-parallel over batch):
  - Each core processes B_LOC = B/8 images; conv weights + BN params replicated.
  - Per-tensor symmetric quantization produces small integers held in bf16
    (ints <= 256 exact); 3x3 convs run as 9 shifted matmuls per c_in group
    accumulating in fp32 PSUM => exact integer arithmetic.
  - Quant scales factor out of batchnorm; all BN math in the integer domain.
  - v2 layout: x is loaded ONCE into SBUF (XY tiles) and never re-read for the
    quantize pass; conv1 output Y1 lives in the same SBUF tiles (x is dead
    after quantize), conv2 output Y2 again reuses them.  The residual is
    prefetched into SBUF during the convs.  No DRAM spill/reload of Y1.
  - Collectives: warmup AllReduce triggered at t~0 (absorbs comm-channel
    init), AllGather everywhere else (shorter mesh than AllReduce); per-group
    stats split [0:12]/[12:16] with conv tail groups [2,1,1] so only a 4-tile
    payload is exposed after the last matmul; collective payload DMAs ride
    the gpsimd queue (never stuck behind bulk traffic on the sync queue).
  - Rounding replicates round-to-nearest-even via the +/- 1.5*2^23 trick.
"""

import numpy as np
from contextlib import ExitStack

import concourse.bass as bass
import concourse.mybir as mybir
import concourse.tile as tile
import concourse.bass_isa as bass_isa
from concourse import bacc
from concourse.bass_utils import run_bass_kernel_spmd

F32 = mybir.dt.float32
BF16 = mybir.dt.bfloat16
AF = mybir.ActivationFunctionType
OP = mybir.AluOpType
AX = mybir.AxisListType

C_MAGIC = 12582912.0  # 1.5 * 2^23 : fp32 add/sub rounds to nearest-even integer
BN_EPS = 1e-5

N_CORES = 8
B = 64          # full batch
C = 256         # channels
H = W = 32
HW = H * W      # 1024
NG = 2          # channel groups of 128
NSP = 2         # spatial halves (16 rows x 32 cols = 512) per image
PHW_ = 34 * 34  # padded image size

_NC_CACHE = {}


def build_nc(b_loc=B // N_CORES, n_cores=N_CORES):
    key = (b_loc, n_cores)
    if key in _NC_CACHE:
        return _NC_CACHE[key]

    nc = bacc.Bacc("TRN2", target_bir_lowering=False, debug=False,
                   num_devices=n_cores)
    groups = [list(range(n_cores))]

    x_in = nc.dram_tensor("x", [b_loc, C, H, W], F32, kind="ExternalInput").ap()
    w1t = nc.dram_tensor("w1t", [9, C, C], F32, kind="ExternalInput").ap()
    w2t = nc.dram_tensor("w2t", [9, C, C], F32, kind="ExternalInput").ap()
    gamma1 = nc.dram_tensor("gamma1", [C], F32, kind="ExternalInput").ap()
    beta1 = nc.dram_tensor("beta1", [C], F32, kind="ExternalInput").ap()
    gamma2 = nc.dram_tensor("gamma2", [C], F32, kind="ExternalInput").ap()
    beta2 = nc.dram_tensor("beta2", [C], F32, kind="ExternalInput").ap()
    out = nc.dram_tensor("out", [b_loc, C, H, W], F32, kind="ExternalOutput").ap()

    wts = [w1t, w2t]
    NT = b_loc * NSP          # psum tiles per c_out group per conv (16)

    with tile.TileContext(nc) as tc, ExitStack() as ctx:
        per = ctx.enter_context(tc.tile_pool(name="persist", bufs=1))
        wf32 = ctx.enter_context(tc.tile_pool(name="wf32", bufs=2))
        zrot = ctx.enter_context(tc.tile_pool(name="zrot", bufs=3))
        orot = ctx.enter_context(tc.tile_pool(name="orot", bufs=2))
        psum = ctx.enter_context(tc.tile_pool(name="psum", bufs=8, space="PSUM"))
        dram = ctx.enter_context(tc.tile_pool(name="dram", bufs=1, space="DRAM"))

        def pt(shape, dtype, name):
            return per.tile(shape, dtype, tag=name, name=name)

        def vts(outap, inap, s1, s2=None, op0=OP.mult, op1=None):
            if op1 is None:
                nc.vector.tensor_scalar(outap, inap, s1, None, op0=op0)
            else:
                nc.vector.tensor_scalar(outap, inap, s1, s2, op0=op0, op1=op1)

        def mk_epse(s_act, s_w, tag):
            """eps / (s_act*s_w)^2 - precomputed off the critical path"""
            se = pt([128, 1], F32, f"se{tag}")
            vts(se[:], s_act[:], s_w[:, 0:1], op0=OP.mult)
            se2 = pt([128, 1], F32, f"se2{tag}")
            vts(se2[:], se[:], se[:, 0:1], op0=OP.mult)
            se2r = pt([128, 1], F32, f"se2r{tag}")
            nc.vector.reciprocal(se2r[:], se2[:])
            epse = pt([128, 1], F32, f"epse{tag}")
            vts(epse[:], se2r[:], float(BN_EPS), op0=OP.mult)
            return epse

        # ---------- warmup collective: very first gpsimd instruction --------
        # AllGather (2 mesh phases, vs 4 for AllReduce) on an uninitialized
        # DRAM tile (pure data movement, result unused) - no producer means
        # no semaphore wait, so the trigger fires the moment the gpsimd
        # engine starts (~10us) and the ~55us comm-channel init + peer-skew
        # wait overlaps the x load / amax / weight-quantize phase.
        wu_in = dram.tile([128], F32, tag="wu_in", name="wu_in")
        wu_out = dram.tile([n_cores, 128], F32, tag="wu_out", name="wu_out")
        nc.gpsimd.collective_compute("AllGather", OP.bypass, replica_groups=groups,
                                     ins=[wu_in.opt()], outs=[wu_out.opt()])

        # padded quantized input tiles; memset on DVE (gpsimd queue must stay
        # free so the warmup collective triggers at t~0)
        xpad = [[None] * b_loc for _ in range(NG)]
        xp3 = [[None] * b_loc for _ in range(NG)]
        for g in range(NG):
            for i in range(b_loc):
                t = pt([128, PHW_], BF16, f"xpad{g}_{i}")
                nc.vector.memset(t[:], 0.0)
                xpad[g][i] = t
                xp3[g][i] = t.rearrange("p (h w) -> p h w", w=34)

        # ---------- constants ----------
        cmag = pt([128, 1], F32, "cmag")
        nc.vector.memset(cmag[:], C_MAGIC)

        # ---------- phase A: x -> SBUF (kept!), local amax -> C0 AllGather --
        XY = [pt([128, (b_loc // 2) * 2048], F32, f"XY{h}") for h in range(2)]

        def xcols(i):      # x image i lives in XY[i//4] cols (i%4)*2048
            return XY[i // (b_loc // 2)], (i % (b_loc // 2)) * 2048

        def ycols(o, i, s):  # conv output (o,i,s) -> XY[o] cols i*1024+s*512
            return XY[o], i * 1024 + s * 512

        xamax = pt([128, b_loc], F32, "xamax")
        for i in range(b_loc):
            xt, c0 = xcols(i)
            nc.sync.dma_start(
                xt[:, c0:c0 + 2048].rearrange("c (g hw) -> c g hw", g=NG),
                x_in[i].rearrange("(g c) h w -> c g (h w)", c=128))
            nc.vector.tensor_reduce(
                xamax[:, i:i + 1],
                xt[:, c0:c0 + 2048].rearrange("c (g hw) -> c g hw", g=NG),
                axis=AX.XY, op=OP.max, apply_absolute_value=True)
        lamax = pt([128, 1], F32, "lamax")
        nc.vector.tensor_reduce(lamax[:], xamax[:], axis=AX.X, op=OP.max)
        # cross-partition max BEFORE the collective (in slack time): the
        # C0 readback then needs no transpose hop on the critical path.
        lat = pt([1, 128], F32, "lat")
        nc.sync.dma_start(lat[:], lamax[:])
        lar = pt([1, 1], F32, "lar")
        nc.vector.tensor_reduce(lar[:], lat[:], axis=AX.X, op=OP.max)
        lap = pt([1, 128], F32, "lap")
        nc.vector.tensor_scalar(lap[:], lat[:], lar[:, 0:1], None, op0=OP.max)
        gxl = pt([128, 1], F32, "gxl")
        nc.sync.dma_start(gxl[:], lap[:])
        cc0_in = dram.tile([128], F32, tag="cc0_in", name="cc0_in")
        cc0_out = dram.tile([n_cores, 128], F32, tag="cc0_out", name="cc0_out")
        nc.gpsimd.dma_start(cc0_in[:].rearrange("(c u) -> c u", u=1), gxl[:])
        nc.gpsimd.collective_compute("AllGather", OP.bypass,
                                     replica_groups=groups,
                                     ins=[cc0_in.opt()], outs=[cc0_out.opt()])
        c0res = pt([128, n_cores], F32, "c0res")
        nc.gpsimd.dma_start(c0res[:], cc0_out[:].rearrange("r c -> c r"))
        gxamax = pt([128, 1], F32, "gxamax")
        nc.vector.tensor_reduce(gxamax[:], c0res[:], axis=AX.X, op=OP.max)
        sx = pt([128, 1], F32, "sx")
        vts(sx[:], gxamax[:], 1.0 / 127.0, 1e-12, op0=OP.mult, op1=OP.add)
        rx = pt([128, 1], F32, "rx")
        nc.vector.reciprocal(rx[:], sx[:])

        # ---------- weights: single load, local amax, quantize ----------
        # wq[conv][g] : [128, 9*256] bf16, block k at k*256
        rw = []
        wq = []
        for ci_, wt in enumerate(wts):
            wfg = []
            wamax = pt([128, NG], F32, f"wamax{ci_}")
            for g in range(NG):
                wf = wf32.tile([128, 9 * C], F32, tag="wf32", name="wf32")
                nc.sync.dma_start(
                    wf[:].rearrange("c (k o) -> c k o", k=9),
                    wt[:, g * 128:(g + 1) * 128, :].rearrange("k c o -> c k o"))
                nc.vector.tensor_reduce(
                    wamax[:, g:g + 1], wf[:], axis=AX.X, op=OP.max,
                    apply_absolute_value=True)
                wfg.append(wf)
            wl = pt([128, 1], F32, f"wlmax{ci_}")
            nc.vector.tensor_reduce(wl[:], wamax[:], axis=AX.X, op=OP.max)
            wlt = pt([1, 128], F32, f"wlt{ci_}")
            nc.sync.dma_start(wlt[:], wl[:])
            wred = pt([1, 1], F32, f"wred{ci_}")
            nc.vector.tensor_reduce(wred[:], wlt[:], axis=AX.X, op=OP.max)
            wrep = pt([1, 128], F32, f"wrep{ci_}")
            nc.vector.tensor_scalar(wrep[:], wlt[:], wred[:, 0:1], None,
                                    op0=OP.max)
            gw = pt([128, 1], F32, f"gwmax{ci_}")
            nc.sync.dma_start(gw[:], wrep[:])
            sw = pt([128, 1], F32, f"sw{ci_}")
            vts(sw[:], gw[:], 1.0 / 127.0, 1e-12, op0=OP.mult, op1=OP.add)
            rwv = pt([128, 1], F32, f"rw{ci_}")
            nc.vector.reciprocal(rwv[:], sw[:])
            rw.append((sw, rwv))
            wqc = []
            WCH = 1152  # quantize in column chunks to keep the pool small
            for g in range(NG):
                wqg = pt([128, 9 * C], BF16, f"wq{ci_}_{g}")
                for c0_ in range(0, 9 * C, WCH):
                    wz = zrot.tile([128, WCH], F32, tag="zrot", name="zrot")
                    nc.scalar.activation(wz[:], wfg[g][:, c0_:c0_ + WCH],
                                         AF.Identity, bias=cmag[:, 0:1],
                                         scale=rwv[:, 0:1])
                    vts(wqg[:, c0_:c0_ + WCH], wz[:], -C_MAGIC, op0=OP.add)
                wqc.append(wqg)
            wq.append(wqc)

        # gamma/beta as [128, o] vectors
        gb = {}
        for name, t in (("g1", gamma1), ("b1", beta1), ("g2", gamma2), ("b2", beta2)):
            v = pt([128, NG], F32, f"gb_{name}")
            for o in range(NG):
                nc.sync.dma_start(
                    v[:, o:o + 1],
                    t[o * 128:(o + 1) * 128].rearrange("(c u) -> c u", u=1))
            gb[name] = v

        epse1 = mk_epse(sx, rw[0][0], "1")  # runs pre-conv, off critical path

        # ---------- phase B: quantize x from SBUF -> xpad (bf16) ----------
        # first two images gate conv1's first matmuls: group 1's scale+bias
        # runs on DVE so ACT and DVE work in parallel.
        for i in range(b_loc):
            xt, c0 = xcols(i)
            for g in range(NG):
                zx = zrot.tile([128, HW], F32, tag="zrot", name="zrot")
                if i < 2 and g == 1:
                    nc.vector.tensor_scalar(zx[:],
                                            xt[:, c0 + g * HW:c0 + (g + 1) * HW],
                                            rx[:, 0:1], cmag[:, 0:1],
                                            op0=OP.mult, op1=OP.add)
                else:
                    nc.scalar.activation(zx[:],
                                         xt[:, c0 + g * HW:c0 + (g + 1) * HW],
                                         AF.Identity, bias=cmag[:, 0:1],
                                         scale=rx[:, 0:1])
                vts(xp3[g][i][:, 1:33, 1:33],
                    zx[:].rearrange("p (h w) -> p h w", w=32), -C_MAGIC,
                    op0=OP.add)

        # residual prefetch for c_out group 0 (sync queue drains under conv1)
        xres = [pt([128, HW], F32, f"xres{i}") for i in range(b_loc)]
        for i in range(b_loc):
            nc.sync.dma_start(xres[i][:], x_in[i, 0:128, :, :])

        # ---------- conv helper ----------
        GT = 4
        # head groups of 2 => the first matmuls wait on image 0 only; tail
        # groups of 2/1/1 => only a small stats payload is exposed post-conv
        TAIL_SIZES = [2, 2, 4, 4, 2, 1, 1]
        STATS_SPLIT = 12

        def conv_group(o, wqc, post_tile, sizes=None, after_cb=None):
            pairs = [(i, s) for i in range(b_loc) for s in range(NSP)]
            if sizes is None:
                sizes = [GT] * (len(pairs) // GT)
            idx = 0
            for sz in sizes:
                grp = pairs[idx:idx + sz]
                idx += sz
                pss = [psum.tile([128, 512], F32, tag="ps", name="ps")
                       for _ in grp]
                for g in range(NG):
                    for k in range(9):
                        ky, kx = divmod(k, 3)
                        first = (g == 0) and (k == 0)
                        last = (g == NG - 1) and (k == 8)
                        wslice = wqc[g][:, k * C + o * 128: k * C + o * 128 + 128]
                        for t, (i, s) in enumerate(grp):
                            nc.tensor.matmul(
                                pss[t][:], wslice,
                                xp3[g][i][:, s * 16 + ky: s * 16 + ky + 16,
                                          kx: kx + 32],
                                start=first, stop=last)
                for t, (i, s) in enumerate(grp):
                    post_tile(i, s, i * NSP + s, pss[t])
                if after_cb is not None:
                    after_cb(idx)

        def all_gather(pay, tagp):
            """AllGather [128,S] -> SBUF view [128, S, n_cores] (strided).
            All DMAs ride the gpsimd queue (in-order with the trigger and
            never stuck behind bulk sync-queue traffic)."""
            S = pay.shape[1]
            cin = dram.tile([128, S], F32, tag=f"cg{tagp}_in", name=f"cg{tagp}_in")
            cout = dram.tile([n_cores, 128, S], F32, tag=f"cg{tagp}_out",
                             name=f"cg{tagp}_out")
            nc.gpsimd.dma_start(cin[:], pay[:])
            nc.gpsimd.collective_compute("AllGather", OP.bypass,
                                         replica_groups=groups,
                                         ins=[cin.opt()], outs=[cout.opt()])
            res = pt([128, n_cores * S], F32, f"cg{tagp}_res")
            nc.gpsimd.dma_start(
                res[:].rearrange("c (r s) -> c r s", s=S),
                cout[:].rearrange("r c s -> c r s"))
            return res.rearrange("c (r s) -> c s r", s=S)

        def bn_coeffs(gsum, epse, gam, bet, tag):
            """global [mean, E[x^2]] sums over cores -> A, B  (t = A*Y + B);
            minimal-op chain (7 ops) since it sits on the critical path."""
            me2 = pt([128, 2], F32, f"me2{tag}")
            vts(me2[:], gsum[:], 1.0 / n_cores, op0=OP.mult)
            negvar = pt([128, 1], F32, f"negvar{tag}")
            nc.vector.scalar_tensor_tensor(
                negvar[:], me2[:, 0:1], me2[:, 0:1], me2[:, 1:2],
                op0=OP.mult, op1=OP.subtract)
            std = pt([128, 1], F32, f"std{tag}")
            nc.scalar.activation(std[:], negvar[:], AF.Sqrt,
                                 bias=epse[:, 0:1], scale=-1.0)
            stdr = pt([128, 1], F32, f"stdr{tag}")
            nc.vector.reciprocal(stdr[:], std[:])
            A = pt([128, 1], F32, f"A{tag}")
            vts(A[:], gam[:], stdr[:, 0:1], op0=OP.mult)
            negmA = pt([128, 1], F32, f"negmA{tag}")
            vts(negmA[:], me2[:, 0:1], A[:, 0:1], -1.0, op0=OP.mult, op1=OP.mult)
            Bv = pt([128, 1], F32, f"B{tag}")
            nc.vector.tensor_add(Bv[:], negmA[:], bet[:])
            return A, Bv

        def stat_payload(pay, pc, bnb, ta, tb, chmx, chmn, tagp):
            """write [mean, E[x^2] (, chmax, -chmin)] over tiles [ta:tb] into
            pay[:, pc:pc+S]"""
            a = pt([128, 2], F32, f"agg{tagp}")
            nc.vector.bn_aggr(a[:], bnb[:, 6 * ta: 6 * tb])
            nc.vector.tensor_copy(pay[:, pc:pc + 1], a[:, 0:1])
            m2 = pt([128, 1], F32, f"m2{tagp}")
            vts(m2[:], a[:, 0:1], a[:, 0:1], op0=OP.mult)
            nc.vector.tensor_add(pay[:, pc + 1:pc + 2], m2[:], a[:, 1:2])
            if chmx is not None:
                nc.vector.tensor_reduce(pay[:, pc + 2:pc + 3], chmx[:, ta:tb],
                                        axis=AX.X, op=OP.max)
                mn = pt([128, 1], F32, f"mn{tagp}")
                nc.vector.tensor_reduce(mn[:], chmn[:, ta:tb],
                                        axis=AX.X, op=OP.min)
                vts(pay[:, pc + 3:pc + 4], mn[:], -1.0, op0=OP.mult)

        def combine_halves(gvA, cA, gvB, cB, with_mx, tagp):
            """weighted sum of [0:12] and [12:16] stat results + max of ranges.
            gvA/gvB are [128, S, n_cores] AG views; cA/cB column offsets."""
            gsA = pt([128, 2], F32, f"gsA{tagp}")
            nc.vector.tensor_reduce(gsA[:], gvA[:, cA:cA + 2, :], axis=AX.X, op=OP.add)
            gsB = pt([128, 2], F32, f"gsB{tagp}")
            nc.vector.tensor_reduce(gsB[:], gvB[:, cB:cB + 2, :], axis=AX.X, op=OP.add)
            wA = pt([128, 2], F32, f"wA{tagp}")
            vts(wA[:], gsA[:], STATS_SPLIT / NT, op0=OP.mult)
            gs = pt([128, 2], F32, f"gs{tagp}")
            nc.vector.scalar_tensor_tensor(
                gs[:], gsB[:], (NT - STATS_SPLIT) / NT, wA[:],
                op0=OP.mult, op1=OP.add)
            gm = None
            if with_mx:
                gmA = pt([128, 2], F32, f"gmA{tagp}")
                nc.vector.tensor_reduce(gmA[:], gvA[:, cA + 2:cA + 4, :],
                                        axis=AX.X, op=OP.max)
                gmB = pt([128, 2], F32, f"gmB{tagp}")
                nc.vector.tensor_reduce(gmB[:], gvB[:, cB + 2:cB + 4, :],
                                        axis=AX.X, op=OP.max)
                gm = pt([128, 2], F32, f"gm{tagp}")
                nc.vector.tensor_max(gm[:], gmA[:], gmB[:])
            return gs, gm

        # ---------- phase C: conv1 ----------
        # Stats collectives: o=0's full-group stats ride the same AllGather
        # as o=1's [0:12] partial (one merged [128,8] payload, fired 12/16
        # through conv1-o1); only the small [12:16] payload is exposed after
        # the last matmul.
        bnb1 = [pt([128, 6 * NT], F32, f"bnb1_{o}") for o in range(NG)]
        chmx1 = [pt([128, NT], F32, f"chmx1_{o}") for o in range(NG)]
        chmn1 = [pt([128, NT], F32, f"chmn1_{o}") for o in range(NG)]
        pay1m = pt([128, 8], F32, "pay1m")
        pay1b = pt([128, 4], F32, "pay1b")

        def mk_post1(o):
            bnb, chmx, chmn = bnb1[o], chmx1[o], chmn1[o]

            def post1(i, s, t, ps):
                yt, yc = ycols(o, i, s)
                nc.scalar.copy(yt[:, yc:yc + 512], ps[:])
                nc.vector.bn_stats(bnb[:, 6 * t: 6 * t + 6], ps[:])
                nc.vector.tensor_reduce(chmx[:, t:t + 1], ps[:], axis=AX.X, op=OP.max)
                nc.vector.tensor_reduce(chmn[:, t:t + 1], ps[:], axis=AX.X, op=OP.min)
            return post1

        conv_group(0, wq[0], mk_post1(0), sizes=TAIL_SIZES)
        stat_payload(pay1m, 0, bnb1[0], 0, NT, chmx1[0], chmn1[0], "1_0")
        gv1m_box = []

        def after1(idx):
            if idx == STATS_SPLIT:
                stat_payload(pay1m, 4, bnb1[1], 0, STATS_SPLIT,
                             chmx1[1], chmn1[1], "1_1a")
                gv1m_box.append(all_gather(pay1m, "1m"))

        conv_group(1, wq[0], mk_post1(1), sizes=TAIL_SIZES, after_cb=after1)
        stat_payload(pay1b, 0, bnb1[1], STATS_SPLIT, NT, chmx1[1], chmn1[1], "1_1b")
        gv1b = all_gather(pay1b, "1b")
        gv1m = gv1m_box[0]

        A1, B1, tmx = [], [], []
        for o in range(NG):
            if o == 0:
                gs = pt([128, 2], F32, "gs1_0")
                nc.vector.tensor_reduce(gs[:], gv1m[:, 0:2, :], axis=AX.X, op=OP.add)
                gm = pt([128, 2], F32, "gm1_0")
                nc.vector.tensor_reduce(gm[:], gv1m[:, 2:4, :], axis=AX.X, op=OP.max)
            else:
                gs, gm = combine_halves(gv1m, 4, gv1b, 0, True, "1_1")
            a_, b_ = bn_coeffs(gs, epse1, gb["g1"][:, o:o + 1],
                               gb["b1"][:, o:o + 1], f"1_{o}")
            A1.append(a_)
            B1.append(b_)
            c1 = pt([128, 1], F32, f"c1_{o}")
            vts(c1[:], gm[:, 0:1], a_[:, 0:1], b_[:, 0:1], op0=OP.mult, op1=OP.add)
            mnv = pt([128, 1], F32, f"mnv_{o}")
            vts(mnv[:], gm[:, 1:2], -1.0, op0=OP.mult)
            c2 = pt([128, 1], F32, f"c2_{o}")
            vts(c2[:], mnv[:], a_[:, 0:1], b_[:, 0:1], op0=OP.mult, op1=OP.add)
            tm = pt([128, 1], F32, f"tmx_{o}")
            nc.vector.tensor_max(tm[:], c1[:], c2[:])
            tmx.append(tm)

        # ---------- phase D: unsigned quant scale ----------
        # (relu clamp folded into the tgp broadcast; cross-partition max via
        # the DMA-transpose trick)
        tmall = pt([128, 1], F32, "tmall")
        nc.vector.tensor_max(tmall[:], tmx[0][:], tmx[1][:])
        tgt = pt([1, 128], F32, "tgt")
        nc.sync.dma_start(tgt[:], tmall[:])
        tgr = pt([1, 1], F32, "tgr")
        nc.vector.tensor_reduce(tgr[:], tgt[:], axis=AX.X, op=OP.max)
        tgp = pt([1, 128], F32, "tgp")
        nc.vector.tensor_scalar(tgp[:], tgt[:], tgr[:, 0:1], 0.0,
                                op0=OP.max, op1=OP.max)
        tg = pt([128, 1], F32, "tg")
        nc.sync.dma_start(tg[:], tgp[:])
        s2q = pt([128, 1], F32, "s2q")
        vts(s2q[:], tg[:], 1.0 / 255.0, 1e-12, op0=OP.mult, op1=OP.add)
        r2q = pt([128, 1], F32, "r2q")
        nc.vector.reciprocal(r2q[:], s2q[:])
        A1p, B1C = [], []
        for o in range(NG):
            ap_ = pt([128, 1], F32, f"A1p_{o}")
            vts(ap_[:], A1[o][:], r2q[:, 0:1], op0=OP.mult)
            bp_ = pt([128, 1], F32, f"B1C_{o}")
            vts(bp_[:], B1[o][:], r2q[:, 0:1], C_MAGIC, op0=OP.mult, op1=OP.add)
            A1p.append(ap_)
            B1C.append(bp_)

        # ---------- phase E: quantize Y1 (SBUF) -> xpad ----------
        # q = relu(round(A1p*Y + B1p)): one fused scale+bias(+C) op, then one
        # DVE op (-C with relu) straight into the bf16 xpad interior.  For
        # the first two images (which gate conv2's first matmuls) group 1's
        # scale+bias runs on DVE so ACT and DVE work in parallel.
        for i in range(b_loc):
            for g in range(NG):
                yt, yc = ycols(g, i, 0)
                z1 = zrot.tile([128, HW], F32, tag="zrot", name="zrot")
                if i < 2 and g == 1:
                    nc.vector.tensor_scalar(z1[:], yt[:, yc:yc + HW],
                                            A1p[g][:, 0:1], B1C[g][:, 0:1],
                                            op0=OP.mult, op1=OP.add)
                else:
                    nc.scalar.activation(z1[:], yt[:, yc:yc + HW], AF.Identity,
                                         bias=B1C[g][:, 0:1], scale=A1p[g][:, 0:1])
                nc.vector.tensor_scalar(
                    xp3[g][i][:, 1:33, 1:33],
                    z1[:].rearrange("p (h w) -> p h w", w=32),
                    -C_MAGIC, 0.0, op0=OP.add, op1=OP.max)

        epse2 = mk_epse(s2q, rw[1][0], "2")  # hidden under conv2

        # ---------- phase F/G/H: conv2 per group + BN2 + final epilogue ------
        for o in range(NG):
            bnb = pt([128, 6 * NT], F32, f"bnb2_{o}")

            def post2(i, s, t, ps, bnb=bnb, o=o):
                yt, yc = ycols(o, i, s)
                nc.scalar.copy(yt[:, yc:yc + 512], ps[:])
                nc.vector.bn_stats(bnb[:, 6 * t: 6 * t + 6], ps[:])

            if o == 0:
                conv_group(o, wq[1], post2, sizes=TAIL_SIZES)
                pay = pt([128, 2], F32, "pay2_0")
                stat_payload(pay, 0, bnb, 0, NT, None, None, "2_0")
                gv = all_gather(pay, "2_0")
                gs2 = pt([128, 2], F32, f"gs2_{o}")
                nc.vector.tensor_reduce(gs2[:], gv[:, 0:2, :], axis=AX.X, op=OP.add)
            else:
                # residual prefetch for group 1 reuses the xres tiles (WAR
                # with the group-0 epilogue reads; loads drain under conv2)
                for i in range(b_loc):
                    nc.sync.dma_start(xres[i][:], x_in[i, 128:256, :, :])
                payA = pt([128, 2], F32, "pay2_1a")
                payB = pt([128, 2], F32, "pay2_1b")
                parts = []

                def after2(idx, bnb=bnb, parts=parts):
                    if idx == STATS_SPLIT:
                        stat_payload(payA, 0, bnb, 0, STATS_SPLIT, None, None, "2_1a")
                        parts.append(all_gather(payA, "2_1a"))

                conv_group(o, wq[1], post2, sizes=TAIL_SIZES, after_cb=after2)
                stat_payload(payB, 0, bnb, STATS_SPLIT, NT, None, None, "2_1b")
                gvB = all_gather(payB, "2_1b")
                gs2, _ = combine_halves(parts[0], 0, gvB, 0, False, "2_1")

            A2, B2 = bn_coeffs(gs2, epse2, gb["g2"][:, o:o + 1],
                               gb["b2"][:, o:o + 1], f"2_{o}")
            # final: relu(A2*Y2 + B2 + x), one [128,1024] tile per image.
            # The A2*Y2+x step writes XY in place (no staging buffer, no
            # pool-recycle stalls); in the fully-exposed last group the
            # gpsimd engine takes images 4-7 so DVE only serializes 4 ops.
            for i in range(b_loc):
                yt, yc = ycols(o, i, 0)
                eng = nc.vector  # gpsimd stt broke neuronxcc lowering
                eng.scalar_tensor_tensor(
                    yt[:, yc:yc + HW], yt[:, yc:yc + HW], A2[:, 0:1],
                    xres[i][:], op0=OP.mult, op1=OP.add)
                osb = orot.tile([128, HW], F32, tag="orot", name="orot")
                nc.scalar.activation(osb[:], yt[:, yc:yc + HW], AF.Relu,
                                     bias=B2[:, 0:1], scale=1.0)
                nc.sync.dma_start(out[i, o * 128:(o + 1) * 128, :, :], osb[:])

    nc.compile()
    _NC_CACHE[key] = nc
    return nc


def _prep_host(x, w1, w2, gamma1, beta1, gamma2, beta2, n_cores):
    w1t = np.ascontiguousarray(
        np.transpose(np.asarray(w1, np.float32), (2, 3, 1, 0)).reshape(9, C, C))
    w2t = np.ascontiguousarray(
        np.transpose(np.asarray(w2, np.float32), (2, 3, 1, 0)).reshape(9, C, C))
    x = np.ascontiguousarray(np.asarray(x, np.float32))
    b_loc = x.shape[0] // n_cores
    in_maps = []
    for c in range(n_cores):
        in_maps.append({
            "x": x[c * b_loc:(c + 1) * b_loc],
            "w1t": w1t, "w2t": w2t,
            "gamma1": np.asarray(gamma1, np.float32),
            "beta1": np.asarray(beta1, np.float32),
            "gamma2": np.asarray(gamma2, np.float32),
            "beta2": np.asarray(beta2, np.float32),
        })
    return in_maps, b_loc


def kernel(x, w1, gamma1, beta1, w2, gamma2, beta2, _trace=False):
    in_maps, b_loc = _prep_host(x, w1, w2, gamma1, beta1, gamma2, beta2, N_CORES)
    nc = build_nc(b_loc, N_CORES)
    res = run_bass_kernel_spmd(nc, in_maps, list(range(N_CORES)), trace=_trace)
    out = np.concatenate(
        [np.asarray(res.results[c]["out"]).reshape(b_loc, C, H, W)
         for c in range(N_CORES)], axis=0)
    if _trace:
        kernel._last_results = res
    return out


# revision 25
# speedup vs baseline: 1.1465x; 1.0556x over previous
"""Trainium2 Bass kernel for a quantized ResNet BasicBlock (dense_cnn).

  y = relu(bn2(conv2(uq(relu(bn1(conv1(q(x), q(w1)))))), q(w2)) + x)

Strategy (8 NeuronCores, data-parallel over batch):
  - Each core processes B_LOC = B/8 images; conv weights + BN params replicated.
  - Per-tensor symmetric quantization produces small integers held in bf16
    (ints <= 256 exact); 3x3 convs run as 9 shifted matmuls per c_in group
    accumulating in fp32 PSUM => exact integer arithmetic.
  - Quant scales factor out of batchnorm; all BN math in the integer domain.
  - v2 layout: x is loaded ONCE into SBUF (XY tiles) and never re-read for the
    quantize pass; conv1 output Y1 lives in the same SBUF tiles (x is dead
    after quantize), conv2 output Y2 again reuses them.  The residual is
    prefetched into SBUF during the convs.  No DRAM spill/reload of Y1.
  - Collectives: warmup AllReduce triggered at t~0 (absorbs comm-channel
    init), AllGather everywhere else (shorter mesh than AllReduce); per-group
    stats split [0:12]/[12:16] with conv tail groups [2,1,1] so only a 4-tile
    payload is exposed after the last matmul; collective payload DMAs ride
    the gpsimd queue (never stuck behind bulk traffic on the sync queue).
  - Rounding replicates round-to-nearest-even via the +/- 1.5*2^23 trick.
"""

import numpy as np
from contextlib import ExitStack

import concourse.bass as bass
import concourse.mybir as mybir
import concourse.tile as tile
import concourse.bass_isa as bass_isa
from concourse import bacc
from concourse.bass_utils import run_bass_kernel_spmd

F32 = mybir.dt.float32
BF16 = mybir.dt.bfloat16
AF = mybir.ActivationFunctionType
OP = mybir.AluOpType
AX = mybir.AxisListType

C_MAGIC = 12582912.0  # 1.5 * 2^23 : fp32 add/sub rounds to nearest-even integer
BN_EPS = 1e-5

N_CORES = 8
B = 64          # full batch
C = 256         # channels
H = W = 32
HW = H * W      # 1024
NG = 2          # channel groups of 128
NSP = 2         # spatial halves (16 rows x 32 cols = 512) per image
PHW_ = 34 * 34  # padded image size

_NC_CACHE = {}


def build_nc(b_loc=B // N_CORES, n_cores=N_CORES):
    key = (b_loc, n_cores)
    if key in _NC_CACHE:
        return _NC_CACHE[key]

    nc = bacc.Bacc("TRN2", target_bir_lowering=False, debug=False,
                   num_devices=n_cores)
    groups = [list(range(n_cores))]

    x_in = nc.dram_tensor("x", [b_loc, C, H, W], F32, kind="ExternalInput").ap()
    w1t = nc.dram_tensor("w1t", [9, C, C], F32, kind="ExternalInput").ap()
    w2t = nc.dram_tensor("w2t", [9, C, C], F32, kind="ExternalInput").ap()
    gamma1 = nc.dram_tensor("gamma1", [C], F32, kind="ExternalInput").ap()
    beta1 = nc.dram_tensor("beta1", [C], F32, kind="ExternalInput").ap()
    gamma2 = nc.dram_tensor("gamma2", [C], F32, kind="ExternalInput").ap()
    beta2 = nc.dram_tensor("beta2", [C], F32, kind="ExternalInput").ap()
    out = nc.dram_tensor("out", [b_loc, C, H, W], F32, kind="ExternalOutput").ap()

    wts = [w1t, w2t]
    NT = b_loc * NSP          # psum tiles per c_out group per conv (16)

    with tile.TileContext(nc) as tc, ExitStack() as ctx:
        per = ctx.enter_context(tc.tile_pool(name="persist", bufs=1))
        wf32 = ctx.enter_context(tc.tile_pool(name="wf32", bufs=2))
        zrot = ctx.enter_context(tc.tile_pool(name="zrot", bufs=3))
        orot = ctx.enter_context(tc.tile_pool(name="orot", bufs=4))
        psum = ctx.enter_context(tc.tile_pool(name="psum", bufs=8, space="PSUM"))
        dram = ctx.enter_context(tc.tile_pool(name="dram", bufs=1, space="DRAM"))

        def pt(shape, dtype, name):
            return per.tile(shape, dtype, tag=name, name=name)

        def vts(outap, inap, s1, s2=None, op0=OP.mult, op1=None):
            if op1 is None:
                nc.vector.tensor_scalar(outap, inap, s1, None, op0=op0)
            else:
                nc.vector.tensor_scalar(outap, inap, s1, s2, op0=op0, op1=op1)

        def mk_epse(s_act, s_w, tag):
            """eps / (s_act*s_w)^2 - precomputed off the critical path"""
            se = pt([128, 1], F32, f"se{tag}")
            vts(se[:], s_act[:], s_w[:, 0:1], op0=OP.mult)
            se2 = pt([128, 1], F32, f"se2{tag}")
            vts(se2[:], se[:], se[:, 0:1], op0=OP.mult)
            se2r = pt([128, 1], F32, f"se2r{tag}")
            nc.vector.reciprocal(se2r[:], se2[:])
            epse = pt([128, 1], F32, f"epse{tag}")
            vts(epse[:], se2r[:], float(BN_EPS), op0=OP.mult)
            return epse

        # ---------- warmup collective: very first gpsimd instruction --------
        # AllGather (2 mesh phases, vs 4 for AllReduce) on an uninitialized
        # DRAM tile (pure data movement, result unused) - no producer means
        # no semaphore wait, so the trigger fires the moment the gpsimd
        # engine starts (~10us) and the ~55us comm-channel init + peer-skew
        # wait overlaps the x load / amax / weight-quantize phase.
        wu_in = dram.tile([128], F32, tag="wu_in", name="wu_in")
        wu_out = dram.tile([n_cores, 128], F32, tag="wu_out", name="wu_out")
        nc.gpsimd.collective_compute("AllGather", OP.bypass, replica_groups=groups,
                                     ins=[wu_in.opt()], outs=[wu_out.opt()])

        # padded quantized input tiles; memset on DVE (gpsimd queue must stay
        # free so the warmup collective triggers at t~0)
        xpad = [[None] * b_loc for _ in range(NG)]
        xp3 = [[None] * b_loc for _ in range(NG)]
        for g in range(NG):
            for i in range(b_loc):
                t = pt([128, PHW_], BF16, f"xpad{g}_{i}")
                nc.vector.memset(t[:], 0.0)
                xpad[g][i] = t
                xp3[g][i] = t.rearrange("p (h w) -> p h w", w=34)

        # ---------- constants ----------
        cmag = pt([128, 1], F32, "cmag")
        nc.vector.memset(cmag[:], C_MAGIC)

        # ---------- phase A: x -> SBUF (kept!), local amax -> C0 AllGather --
        XY = [pt([128, (b_loc // 2) * 2048], F32, f"XY{h}") for h in range(2)]

        def xcols(i):      # x image i lives in XY[i//4] cols (i%4)*2048
            return XY[i // (b_loc // 2)], (i % (b_loc // 2)) * 2048

        def ycols(o, i, s):  # conv output (o,i,s) -> XY[o] cols i*1024+s*512
            return XY[o], i * 1024 + s * 512

        xamax = pt([128, b_loc], F32, "xamax")
        for i in range(b_loc):
            xt, c0 = xcols(i)
            nc.sync.dma_start(
                xt[:, c0:c0 + 2048].rearrange("c (g hw) -> c g hw", g=NG),
                x_in[i].rearrange("(g c) h w -> c g (h w)", c=128))
            nc.vector.tensor_reduce(
                xamax[:, i:i + 1],
                xt[:, c0:c0 + 2048].rearrange("c (g hw) -> c g hw", g=NG),
                axis=AX.XY, op=OP.max, apply_absolute_value=True)
        lamax = pt([128, 1], F32, "lamax")
        nc.vector.tensor_reduce(lamax[:], xamax[:], axis=AX.X, op=OP.max)
        # cross-partition max BEFORE the collective (in slack time): the
        # C0 readback then needs no transpose hop on the critical path.
        lat = pt([1, 128], F32, "lat")
        nc.sync.dma_start(lat[:], lamax[:])
        lar = pt([1, 1], F32, "lar")
        nc.vector.tensor_reduce(lar[:], lat[:], axis=AX.X, op=OP.max)
        lap = pt([1, 128], F32, "lap")
        nc.vector.tensor_scalar(lap[:], lat[:], lar[:, 0:1], None, op0=OP.max)
        gxl = pt([128, 1], F32, "gxl")
        nc.sync.dma_start(gxl[:], lap[:])
        cc0_in = dram.tile([128], F32, tag="cc0_in", name="cc0_in")
        cc0_out = dram.tile([n_cores, 128], F32, tag="cc0_out", name="cc0_out")
        nc.gpsimd.dma_start(cc0_in[:].rearrange("(c u) -> c u", u=1), gxl[:])
        nc.gpsimd.collective_compute("AllGather", OP.bypass,
                                     replica_groups=groups,
                                     ins=[cc0_in.opt()], outs=[cc0_out.opt()])
        c0res = pt([128, n_cores], F32, "c0res")
        nc.gpsimd.dma_start(c0res[:], cc0_out[:].rearrange("r c -> c r"))
        gxamax = pt([128, 1], F32, "gxamax")
        nc.vector.tensor_reduce(gxamax[:], c0res[:], axis=AX.X, op=OP.max)
        sx = pt([128, 1], F32, "sx")
        vts(sx[:], gxamax[:], 1.0 / 127.0, 1e-12, op0=OP.mult, op1=OP.add)
        rx = pt([128, 1], F32, "rx")
        nc.vector.reciprocal(rx[:], sx[:])

        # ---------- weights: single load, local amax, quantize ----------
        # wq[conv][g] : [128, 9*256] bf16, block k at k*256
        rw = []
        wq = []
        for ci_, wt in enumerate(wts):
            wfg = []
            wamax = pt([128, NG], F32, f"wamax{ci_}")
            for g in range(NG):
                wf = wf32.tile([128, 9 * C], F32, tag="wf32", name="wf32")
                nc.sync.dma_start(
                    wf[:].rearrange("c (k o) -> c k o", k=9),
                    wt[:, g * 128:(g + 1) * 128, :].rearrange("k c o -> c k o"))
                nc.vector.tensor_reduce(
                    wamax[:, g:g + 1], wf[:], axis=AX.X, op=OP.max,
                    apply_absolute_value=True)
                wfg.append(wf)
            wl = pt([128, 1], F32, f"wlmax{ci_}")
            nc.vector.tensor_reduce(wl[:], wamax[:], axis=AX.X, op=OP.max)
            wlt = pt([1, 128], F32, f"wlt{ci_}")
            nc.sync.dma_start(wlt[:], wl[:])
            wred = pt([1, 1], F32, f"wred{ci_}")
            nc.vector.tensor_reduce(wred[:], wlt[:], axis=AX.X, op=OP.max)
            wrep = pt([1, 128], F32, f"wrep{ci_}")
            nc.vector.tensor_scalar(wrep[:], wlt[:], wred[:, 0:1], None,
                                    op0=OP.max)
            gw = pt([128, 1], F32, f"gwmax{ci_}")
            nc.sync.dma_start(gw[:], wrep[:])
            sw = pt([128, 1], F32, f"sw{ci_}")
            vts(sw[:], gw[:], 1.0 / 127.0, 1e-12, op0=OP.mult, op1=OP.add)
            rwv = pt([128, 1], F32, f"rw{ci_}")
            nc.vector.reciprocal(rwv[:], sw[:])
            rw.append((sw, rwv))
            wqc = []
            WCH = 1152  # quantize in column chunks to keep the pool small
            for g in range(NG):
                wqg = pt([128, 9 * C], BF16, f"wq{ci_}_{g}")
                for c0_ in range(0, 9 * C, WCH):
                    wz = zrot.tile([128, WCH], F32, tag="zrot", name="zrot")
                    nc.scalar.activation(wz[:], wfg[g][:, c0_:c0_ + WCH],
                                         AF.Identity, bias=cmag[:, 0:1],
                                         scale=rwv[:, 0:1])
                    vts(wqg[:, c0_:c0_ + WCH], wz[:], -C_MAGIC, op0=OP.add)
                wqc.append(wqg)
            wq.append(wqc)

        # gamma/beta as [128, o] vectors
        gb = {}
        for name, t in (("g1", gamma1), ("b1", beta1), ("g2", gamma2), ("b2", beta2)):
            v = pt([128, NG], F32, f"gb_{name}")
            for o in range(NG):
                nc.sync.dma_start(
                    v[:, o:o + 1],
                    t[o * 128:(o + 1) * 128].rearrange("(c u) -> c u", u=1))
            gb[name] = v

        epse1 = mk_epse(sx, rw[0][0], "1")  # runs pre-conv, off critical path

        # ---------- phase B: quantize x from SBUF -> xpad (bf16) ----------
        # first two images gate conv1's first matmuls: group 1's scale+bias
        # runs on DVE so ACT and DVE work in parallel.
        for i in range(b_loc):
            xt, c0 = xcols(i)
            for g in range(NG):
                zx = zrot.tile([128, HW], F32, tag="zrot", name="zrot")
                if i < 2 and g == 1:
                    nc.vector.tensor_scalar(zx[:],
                                            xt[:, c0 + g * HW:c0 + (g + 1) * HW],
                                            rx[:, 0:1], cmag[:, 0:1],
                                            op0=OP.mult, op1=OP.add)
                else:
                    nc.scalar.activation(zx[:],
                                         xt[:, c0 + g * HW:c0 + (g + 1) * HW],
                                         AF.Identity, bias=cmag[:, 0:1],
                                         scale=rx[:, 0:1])
                vts(xp3[g][i][:, 1:33, 1:33],
                    zx[:].rearrange("p (h w) -> p h w", w=32), -C_MAGIC,
                    op0=OP.add)

        # residual prefetch for c_out group 0 (sync queue drains under conv1)
        xres = [pt([128, HW], F32, f"xres{i}") for i in range(b_loc)]
        for i in range(b_loc):
            nc.sync.dma_start(xres[i][:], x_in[i, 0:128, :, :])

        # ---------- conv helper ----------
        GT = 4
        # head groups of 2 => the first matmuls wait on image 0 only; tail
        # groups of 2/1/1 => only a small stats payload is exposed post-conv
        TAIL_SIZES = [2, 2, 4, 4, 2, 1, 1]
        STATS_SPLIT = 12

        def conv_group(o, wqc, post_tile, sizes=None, after_cb=None):
            pairs = [(i, s) for i in range(b_loc) for s in range(NSP)]
            if sizes is None:
                sizes = [GT] * (len(pairs) // GT)
            idx = 0
            for sz in sizes:
                grp = pairs[idx:idx + sz]
                idx += sz
                pss = [psum.tile([128, 512], F32, tag="ps", name="ps")
                       for _ in grp]
                for g in range(NG):
                    for k in range(9):
                        ky, kx = divmod(k, 3)
                        first = (g == 0) and (k == 0)
                        last = (g == NG - 1) and (k == 8)
                        wslice = wqc[g][:, k * C + o * 128: k * C + o * 128 + 128]
                        for t, (i, s) in enumerate(grp):
                            nc.tensor.matmul(
                                pss[t][:], wslice,
                                xp3[g][i][:, s * 16 + ky: s * 16 + ky + 16,
                                          kx: kx + 32],
                                start=first, stop=last)
                for t, (i, s) in enumerate(grp):
                    post_tile(i, s, i * NSP + s, pss[t])
                if after_cb is not None:
                    after_cb(idx)

        def all_gather(pay, tagp):
            """AllGather [128,S] -> SBUF view [128, S, n_cores] (strided).
            All DMAs ride the gpsimd queue (in-order with the trigger and
            never stuck behind bulk sync-queue traffic)."""
            S = pay.shape[1]
            cin = dram.tile([128, S], F32, tag=f"cg{tagp}_in", name=f"cg{tagp}_in")
            cout = dram.tile([n_cores, 128, S], F32, tag=f"cg{tagp}_out",
                             name=f"cg{tagp}_out")
            nc.gpsimd.dma_start(cin[:], pay[:])
            nc.gpsimd.collective_compute("AllGather", OP.bypass,
                                         replica_groups=groups,
                                         ins=[cin.opt()], outs=[cout.opt()])
            res = pt([128, n_cores * S], F32, f"cg{tagp}_res")
            nc.gpsimd.dma_start(
                res[:].rearrange("c (r s) -> c r s", s=S),
                cout[:].rearrange("r c s -> c r s"))
            return res.rearrange("c (r s) -> c s r", s=S)

        def bn_coeffs(gsum, epse, gam, bet, tag):
            """global [mean, E[x^2]] sums over cores -> A, B  (t = A*Y + B);
            minimal-op chain (7 ops) since it sits on the critical path."""
            me2 = pt([128, 2], F32, f"me2{tag}")
            vts(me2[:], gsum[:], 1.0 / n_cores, op0=OP.mult)
            negvar = pt([128, 1], F32, f"negvar{tag}")
            nc.vector.scalar_tensor_tensor(
                negvar[:], me2[:, 0:1], me2[:, 0:1], me2[:, 1:2],
                op0=OP.mult, op1=OP.subtract)
            std = pt([128, 1], F32, f"std{tag}")
            nc.scalar.activation(std[:], negvar[:], AF.Sqrt,
                                 bias=epse[:, 0:1], scale=-1.0)
            stdr = pt([128, 1], F32, f"stdr{tag}")
            nc.vector.reciprocal(stdr[:], std[:])
            A = pt([128, 1], F32, f"A{tag}")
            vts(A[:], gam[:], stdr[:, 0:1], op0=OP.mult)
            negmA = pt([128, 1], F32, f"negmA{tag}")
            vts(negmA[:], me2[:, 0:1], A[:, 0:1], -1.0, op0=OP.mult, op1=OP.mult)
            Bv = pt([128, 1], F32, f"B{tag}")
            nc.vector.tensor_add(Bv[:], negmA[:], bet[:])
            return A, Bv

        def stat_payload(pay, pc, bnb, ta, tb, chmx, chmn, tagp):
            """write [mean, E[x^2] (, chmax, -chmin)] over tiles [ta:tb] into
            pay[:, pc:pc+S]"""
            a = pt([128, 2], F32, f"agg{tagp}")
            nc.vector.bn_aggr(a[:], bnb[:, 6 * ta: 6 * tb])
            nc.vector.tensor_copy(pay[:, pc:pc + 1], a[:, 0:1])
            m2 = pt([128, 1], F32, f"m2{tagp}")
            vts(m2[:], a[:, 0:1], a[:, 0:1], op0=OP.mult)
            nc.vector.tensor_add(pay[:, pc + 1:pc + 2], m2[:], a[:, 1:2])
            if chmx is not None:
                nc.vector.tensor_reduce(pay[:, pc + 2:pc + 3], chmx[:, ta:tb],
                                        axis=AX.X, op=OP.max)
                mn = pt([128, 1], F32, f"mn{tagp}")
                nc.vector.tensor_reduce(mn[:], chmn[:, ta:tb],
                                        axis=AX.X, op=OP.min)
                vts(pay[:, pc + 3:pc + 4], mn[:], -1.0, op0=OP.mult)

        def combine_halves(gvA, cA, gvB, cB, with_mx, tagp):
            """weighted sum of [0:12] and [12:16] stat results + max of ranges.
            gvA/gvB are [128, S, n_cores] AG views; cA/cB column offsets."""
            gsA = pt([128, 2], F32, f"gsA{tagp}")
            nc.vector.tensor_reduce(gsA[:], gvA[:, cA:cA + 2, :], axis=AX.X, op=OP.add)
            gsB = pt([128, 2], F32, f"gsB{tagp}")
            nc.vector.tensor_reduce(gsB[:], gvB[:, cB:cB + 2, :], axis=AX.X, op=OP.add)
            wA = pt([128, 2], F32, f"wA{tagp}")
            vts(wA[:], gsA[:], STATS_SPLIT / NT, op0=OP.mult)
            gs = pt([128, 2], F32, f"gs{tagp}")
            nc.vector.scalar_tensor_tensor(
                gs[:], gsB[:], (NT - STATS_SPLIT) / NT, wA[:],
                op0=OP.mult, op1=OP.add)
            gm = None
            if with_mx:
                gmA = pt([128, 2], F32, f"gmA{tagp}")
                nc.vector.tensor_reduce(gmA[:], gvA[:, cA + 2:cA + 4, :],
                                        axis=AX.X, op=OP.max)
                gmB = pt([128, 2], F32, f"gmB{tagp}")
                nc.vector.tensor_reduce(gmB[:], gvB[:, cB + 2:cB + 4, :],
                                        axis=AX.X, op=OP.max)
                gm = pt([128, 2], F32, f"gm{tagp}")
                nc.vector.tensor_max(gm[:], gmA[:], gmB[:])
            return gs, gm

        # ---------- phase C: conv1 ----------
        # Stats collectives: o=0's full-group stats ride the same AllGather
        # as o=1's [0:12] partial (one merged [128,8] payload, fired 12/16
        # through conv1-o1); only the small [12:16] payload is exposed after
        # the last matmul.
        bnb1 = [pt([128, 6 * NT], F32, f"bnb1_{o}") for o in range(NG)]
        chmx1 = [pt([128, NT], F32, f"chmx1_{o}") for o in range(NG)]
        chmn1 = [pt([128, NT], F32, f"chmn1_{o}") for o in range(NG)]
        pay1m = pt([128, 8], F32, "pay1m")
        pay1b = pt([128, 4], F32, "pay1b")

        def mk_post1(o):
            bnb, chmx, chmn = bnb1[o], chmx1[o], chmn1[o]

            def post1(i, s, t, ps):
                yt, yc = ycols(o, i, s)
                nc.scalar.copy(yt[:, yc:yc + 512], ps[:])
                nc.vector.bn_stats(bnb[:, 6 * t: 6 * t + 6], ps[:])
                nc.vector.tensor_reduce(chmx[:, t:t + 1], ps[:], axis=AX.X, op=OP.max)
                nc.vector.tensor_reduce(chmn[:, t:t + 1], ps[:], axis=AX.X, op=OP.min)
            return post1

        conv_group(0, wq[0], mk_post1(0), sizes=TAIL_SIZES)
        stat_payload(pay1m, 0, bnb1[0], 0, NT, chmx1[0], chmn1[0], "1_0")
        gv1m_box = []

        def after1(idx):
            if idx == STATS_SPLIT:
                stat_payload(pay1m, 4, bnb1[1], 0, STATS_SPLIT,
                             chmx1[1], chmn1[1], "1_1a")
                gv1m_box.append(all_gather(pay1m, "1m"))

        conv_group(1, wq[0], mk_post1(1), sizes=TAIL_SIZES, after_cb=after1)
        stat_payload(pay1b, 0, bnb1[1], STATS_SPLIT, NT, chmx1[1], chmn1[1], "1_1b")
        gv1b = all_gather(pay1b, "1b")
        gv1m = gv1m_box[0]

        A1, B1, tmx = [], [], []
        for o in range(NG):
            if o == 0:
                gs = pt([128, 2], F32, "gs1_0")
                nc.vector.tensor_reduce(gs[:], gv1m[:, 0:2, :], axis=AX.X, op=OP.add)
                gm = pt([128, 2], F32, "gm1_0")
                nc.vector.tensor_reduce(gm[:], gv1m[:, 2:4, :], axis=AX.X, op=OP.max)
            else:
                gs, gm = combine_halves(gv1m, 4, gv1b, 0, True, "1_1")
            a_, b_ = bn_coeffs(gs, epse1, gb["g1"][:, o:o + 1],
                               gb["b1"][:, o:o + 1], f"1_{o}")
            A1.append(a_)
            B1.append(b_)
            c1 = pt([128, 1], F32, f"c1_{o}")
            vts(c1[:], gm[:, 0:1], a_[:, 0:1], b_[:, 0:1], op0=OP.mult, op1=OP.add)
            mnv = pt([128, 1], F32, f"mnv_{o}")
            vts(mnv[:], gm[:, 1:2], -1.0, op0=OP.mult)
            c2 = pt([128, 1], F32, f"c2_{o}")
            vts(c2[:], mnv[:], a_[:, 0:1], b_[:, 0:1], op0=OP.mult, op1=OP.add)
            tm = pt([128, 1], F32, f"tmx_{o}")
            nc.vector.tensor_max(tm[:], c1[:], c2[:])
            tmx.append(tm)

        # ---------- phase D: unsigned quant scale ----------
        # (relu clamp folded into the tgp broadcast; cross-partition max via
        # the DMA-transpose trick)
        tmall = pt([128, 1], F32, "tmall")
        nc.vector.tensor_max(tmall[:], tmx[0][:], tmx[1][:])
        tgt = pt([1, 128], F32, "tgt")
        nc.sync.dma_start(tgt[:], tmall[:])
        tgr = pt([1, 1], F32, "tgr")
        nc.vector.tensor_reduce(tgr[:], tgt[:], axis=AX.X, op=OP.max)
        tgp = pt([1, 128], F32, "tgp")
        nc.vector.tensor_scalar(tgp[:], tgt[:], tgr[:, 0:1], 0.0,
                                op0=OP.max, op1=OP.max)
        tg = pt([128, 1], F32, "tg")
        nc.sync.dma_start(tg[:], tgp[:])
        s2q = pt([128, 1], F32, "s2q")
        vts(s2q[:], tg[:], 1.0 / 255.0, 1e-12, op0=OP.mult, op1=OP.add)
        r2q = pt([128, 1], F32, "r2q")
        nc.vector.reciprocal(r2q[:], s2q[:])
        A1p, B1C = [], []
        for o in range(NG):
            ap_ = pt([128, 1], F32, f"A1p_{o}")
            vts(ap_[:], A1[o][:], r2q[:, 0:1], op0=OP.mult)
            bp_ = pt([128, 1], F32, f"B1C_{o}")
            vts(bp_[:], B1[o][:], r2q[:, 0:1], C_MAGIC, op0=OP.mult, op1=OP.add)
            A1p.append(ap_)
            B1C.append(bp_)

        # ---------- phase E: quantize Y1 (SBUF) -> xpad ----------
        # q = relu(round(A1p*Y + B1p)): one fused scale+bias(+C) op, then one
        # DVE op (-C with relu) straight into the bf16 xpad interior.  For
        # the first two images (which gate conv2's first matmuls) group 1's
        # scale+bias runs on DVE so ACT and DVE work in parallel.
        for i in range(b_loc):
            for g in range(NG):
                yt, yc = ycols(g, i, 0)
                z1 = zrot.tile([128, HW], F32, tag="zrot", name="zrot")
                if i < 2 and g == 1:
                    nc.vector.tensor_scalar(z1[:], yt[:, yc:yc + HW],
                                            A1p[g][:, 0:1], B1C[g][:, 0:1],
                                            op0=OP.mult, op1=OP.add)
                else:
                    nc.scalar.activation(z1[:], yt[:, yc:yc + HW], AF.Identity,
                                         bias=B1C[g][:, 0:1], scale=A1p[g][:, 0:1])
                nc.vector.tensor_scalar(
                    xp3[g][i][:, 1:33, 1:33],
                    z1[:].rearrange("p (h w) -> p h w", w=32),
                    -C_MAGIC, 0.0, op0=OP.add, op1=OP.max)

        epse2 = mk_epse(s2q, rw[1][0], "2")  # hidden under conv2

        # ---------- phase F/G/H: conv2 per group + BN2 + final epilogue ------
        for o in range(NG):
            bnb = pt([128, 6 * NT], F32, f"bnb2_{o}")

            def post2(i, s, t, ps, bnb=bnb, o=o):
                yt, yc = ycols(o, i, s)
                nc.scalar.copy(yt[:, yc:yc + 512], ps[:])
                nc.vector.bn_stats(bnb[:, 6 * t: 6 * t + 6], ps[:])

            if o == 0:
                conv_group(o, wq[1], post2, sizes=TAIL_SIZES)
                pay = pt([128, 2], F32, "pay2_0")
                stat_payload(pay, 0, bnb, 0, NT, None, None, "2_0")
                gv = all_gather(pay, "2_0")
                gs2 = pt([128, 2], F32, f"gs2_{o}")
                nc.vector.tensor_reduce(gs2[:], gv[:, 0:2, :], axis=AX.X, op=OP.add)
            else:
                # residual prefetch for group 1 reuses the xres tiles (WAR
                # with the group-0 epilogue reads; loads drain under conv2)
                for i in range(b_loc):
                    nc.sync.dma_start(xres[i][:], x_in[i, 128:256, :, :])
                payA = pt([128, 2], F32, "pay2_1a")
                payB = pt([128, 2], F32, "pay2_1b")
                parts = []

                def after2(idx, bnb=bnb, parts=parts):
                    if idx == STATS_SPLIT:
                        stat_payload(payA, 0, bnb, 0, STATS_SPLIT, None, None, "2_1a")
                        parts.append(all_gather(payA, "2_1a"))

                conv_group(o, wq[1], post2, sizes=TAIL_SIZES, after_cb=after2)
                stat_payload(payB, 0, bnb, STATS_SPLIT, NT, None, None, "2_1b")
                gvB = all_gather(payB, "2_1b")
                gs2, _ = combine_halves(parts[0], 0, gvB, 0, False, "2_1")

            A2, B2 = bn_coeffs(gs2, epse2, gb["g2"][:, o:o + 1],
                               gb["b2"][:, o:o + 1], f"2_{o}")
            # final: relu(A2*Y2 + B2 + x), one [128,1024] tile per image.
            # The A2*Y2+x step writes XY in place (no staging buffer, no
            # pool-recycle stalls); in the fully-exposed last group the
            # gpsimd engine takes images 4-7 so DVE only serializes 4 ops.
            for i in range(b_loc):
                yt, yc = ycols(o, i, 0)
                eng = nc.vector  # gpsimd stt broke neuronxcc lowering
                eng.scalar_tensor_tensor(
                    yt[:, yc:yc + HW], yt[:, yc:yc + HW], A2[:, 0:1],
                    xres[i][:], op0=OP.mult, op1=OP.add)
                osb = orot.tile([128, HW], F32, tag="orot", name="orot")
                nc.scalar.activation(osb[:], yt[:, yc:yc + HW], AF.Relu,
                                     bias=B2[:, 0:1], scale=1.0)
                nc.sync.dma_start(out[i, o * 128:(o + 1) * 128, :, :], osb[:])

    nc.compile()
    _NC_CACHE[key] = nc
    return nc


def _prep_host(x, w1, w2, gamma1, beta1, gamma2, beta2, n_cores):
    w1t = np.ascontiguousarray(
        np.transpose(np.asarray(w1, np.float32), (2, 3, 1, 0)).reshape(9, C, C))
    w2t = np.ascontiguousarray(
        np.transpose(np.asarray(w2, np.float32), (2, 3, 1, 0)).reshape(9, C, C))
    x = np.ascontiguousarray(np.asarray(x, np.float32))
    b_loc = x.shape[0] // n_cores
    in_maps = []
    for c in range(n_cores):
        in_maps.append({
            "x": x[c * b_loc:(c + 1) * b_loc],
            "w1t": w1t, "w2t": w2t,
            "gamma1": np.asarray(gamma1, np.float32),
            "beta1": np.asarray(beta1, np.float32),
            "gamma2": np.asarray(gamma2, np.float32),
            "beta2": np.asarray(beta2, np.float32),
        })
    return in_maps, b_loc


def kernel(x, w1, gamma1, beta1, w2, gamma2, beta2, _trace=False):
    in_maps, b_loc = _prep_host(x, w1, w2, gamma1, beta1, gamma2, beta2, N_CORES)
    nc = build_nc(b_loc, N_CORES)
    res = run_bass_kernel_spmd(nc, in_maps, list(range(N_CORES)), trace=_trace)
    out = np.concatenate(
        [np.asarray(res.results[c]["out"]).reshape(b_loc, C, H, W)
         for c in range(N_CORES)], axis=0)
    if _trace:
        kernel._last_results = res
    return out
